# revision 11
# baseline (speedup 1.0000x reference)
"""Trainium2 Bass kernel for the Inertia model (nn_Net_55224689492388).

Math (exact restructuring of the reference scan; per (row n, channel d)):

  burn-in (t < b):
    app_t = (1 - mask_{t-1}) * mask_t        (mask_{-1} = 0)
    dx_t  = src_t - src_{t-1}                (src_{-1} = 0)
    v_t   = app_t * v_{t-1} + dx_t * (1 - app_t)
    y_t   = src_t + v_t
  post (t >= b): v stays constant (x_t - prev_x collapses to v_{t-1}), so
    y_t   = y_{b-1} + (t - b + 1) * v_{b-1}

Only v is sequential - a first-order linear recurrence computed with the
DVE TensorTensorScan instruction; everything else is bulk elementwise.
The kernel loads only the first b timesteps of src/mask (later steps
cannot affect the output) and writes the full output.

Implementation notes:
- Sign trick: nbt = (app - 1) * dx (one scalar_tensor_tensor op) makes
  the scan compute nv = -v, so y_burn = src - nv and the post phase uses
  a host-provided ramp of -(k+1): y_post = rampneg * nv_b1 + y_b1.
- Scan batching: zeroing app at each sequence's first element makes the
  scan self-initializing (v_0 = 0 * carry + nbt_0), so a single scan
  instruction covers every sequence of a chunk, channel-deinterleaved
  (d-major) intermediates making it one contiguous stride-1 pass.
- The binary mask travels as uint8 (values {0,1} exactly), host-packed
  so each partition's chunk slice is one contiguous 512B DMA run; the
  cast back to f32 rides for free on the Pool's (1 - m) op. Falls back
  to plain f32 mask loads if the mask is ever non-binary.
- Engines: Pool computes (1-m) and app; DVE does dx, nbt, the scan and
  y_burn; ACT does the post-phase extrapolation; all DMA on SP HWDGE.

Sharding: pure data parallel - 65536 rows split as 8192 rows x 8 cores,
no cross-core communication.
"""

import numpy as np

import concourse.bacc as bacc
import concourse.mybir as mybir
from concourse.bass_utils import run_bass_kernel_spmd
from concourse.tile import TileContext

N, T, D = 65536, 128, 2
NCORES = 8
NPART = 128
ROWS_CORE = N // NCORES  # 8192

F32 = mybir.dt.float32
U8 = mybir.dt.uint8
Alu = mybir.AluOpType
Act = mybir.ActivationFunctionType

# Stash of the most recent BassKernelResults (for test.py profiling).
last_results = None


def _build(b, mask_u8=True, g=4, io_g=2, io_bufs=6, wk_bufs=4):
    """Build the per-core Bass module for effective burn-in b (1..T)."""
    G = g
    GIO = io_g * g
    NCHUNK = ROWS_CORE // (NPART * G)
    NBIG = ROWS_CORE // (NPART * GIO)
    post = T - b
    cb = 2 * b  # burn-region columns (t-major, d-interleaved)
    cf = 2 * T  # full row columns

    nc = bacc.Bacc("TRN2", target_bir_lowering=False, debug=False)
    src = nc.dram_tensor("src", [ROWS_CORE, T, D], F32, kind="ExternalInput")
    if mask_u8:
        msk = nc.dram_tensor("msku8", [NBIG, NPART, GIO * cb], U8, kind="ExternalInput")
    else:
        msk = nc.dram_tensor("msk", [ROWS_CORE, T, D], F32, kind="ExternalInput")
    out = nc.dram_tensor("out", [ROWS_CORE, T, D], F32, kind="ExternalOutput")
    if post:
        rampneg = nc.dram_tensor("rampneg", [NPART, post], F32, kind="ExternalInput")

    # row = ci*(128*GIO) + p*GIO + a*G + g : each partition holds GIO
    # consecutive rows, so the output DMA sees large contiguous runs.
    srcv = src[:].rearrange("(c p a g) t d -> c p a g (t d)", p=NPART, a=io_g, g=G)
    outv = out[:].rearrange("(c p a g) t d -> c p a g (t d)", p=NPART, a=io_g, g=G)
    if mask_u8:
        mskv = msk[:].rearrange("c p (a g x) -> c p a g x", a=io_g, g=G)
    else:
        mskv = msk[:].rearrange("(c p a g) t d -> c p a g (t d)", p=NPART, a=io_g, g=G)

    with TileContext(nc) as tc:
        with (
            tc.tile_pool(name="const", bufs=1) as cpool,
            tc.tile_pool(name="io", bufs=io_bufs) as iop,
            tc.tile_pool(name="wk", bufs=wk_bufs) as wkp,
        ):
            if post:
                ramp_t = cpool.tile([NPART, post], F32, name="ramp_t")
                nc.sync.dma_start(out=ramp_t, in_=rampneg[:])

            s_big = m_big = y_big = None
            for c in range(NCHUNK):
                ci, cs = divmod(c, io_g)
                if cs == 0:
                    s_big = iop.tile([NPART, io_g, G, 2 + cb], F32, name="s_ext")
                    m_big = iop.tile(
                        [NPART, io_g, G, cb], U8 if mask_u8 else F32, name="m_t"
                    )
                    y_big = iop.tile([NPART, io_g, G, cf], F32, name="y")
                    nc.vector.memset(s_big[:, :, :, 0:2], 0.0)  # src_{-1} = 0
                    nc.sync.dma_start(
                        out=s_big[:, :, :, 2:], in_=srcv[ci, :, :, :, 0:cb]
                    )
                    if mask_u8:
                        nc.sync.dma_start(out=m_big, in_=mskv[ci])
                    else:
                        nc.sync.dma_start(out=m_big, in_=mskv[ci, :, :, :, 0:cb])
                s_ext = s_big[:, cs]
                m_t = m_big[:, cs]
                y = y_big[:, cs]

                # d-major intermediates: [p, g, d, t]
                omm = wkp.tile([NPART, G, 2, 1 + b], F32, name="omm")
                app = wkp.tile([NPART, G, 2, b], F32, name="app")
                dx = wkp.tile([NPART, G, 2, b], F32, name="dx")
                nbt = wkp.tile([NPART, G, 2, b], F32, name="nbt")
                nv = wkp.tile([NPART, G, 2, b], F32, name="nv")

                m4 = m_t.rearrange("p g (t d) -> p g d t", d=2)
                s_hi4 = s_ext[:, :, 2:].rearrange("p g (t d) -> p g d t", d=2)
                s_lo4 = s_ext[:, :, 0:cb].rearrange("p g (t d) -> p g d t", d=2)

                # omm = 1 - mask (u8 -> f32 cast rides along); lead col = 1
                nc.gpsimd.memset(omm[:, :, :, 0:1], 1.0)
                nc.gpsimd.tensor_scalar(
                    omm[:, :, :, 1:], m4, -1.0, 1.0, Alu.mult, Alu.add
                )
                # app = (1 - mask_{t-1}) * mask_t
                nc.gpsimd.tensor_tensor(app, omm[:, :, :, 0:b], m4, Alu.mult)
                # dx = src_t - src_{t-1}
                nc.vector.tensor_tensor(dx, s_hi4, s_lo4, Alu.subtract)
                # nbt = (app - 1) * dx = -dx*(1-app)
                nc.vector.scalar_tensor_tensor(
                    nbt, app, 1.0, dx, Alu.subtract, Alu.mult
                )
                # self-initializing scan boundaries: zero each sequence's
                # first multiplier (v_0 = 0*carry + nbt_0); after nbt read app.
                nc.vector.memset(app[:, :, :, 0:1], 0.0)

                # single scan across all (g, d) sequences: nv = -v
                nc.vector.tensor_tensor_scan(
                    nv.rearrange("p g d t -> p (g d t)"),
                    app.rearrange("p g d t -> p (g d t)"),
                    nbt.rearrange("p g d t -> p (g d t)"),
                    0.0,
                    Alu.mult,
                    Alu.add,
                )

                # y_burn = src + v = src - nv
                nc.vector.tensor_tensor(
                    y[:, :, 0:cb].rearrange("p g (t d) -> p g t d", d=2),
                    s_ext[:, :, 2:].rearrange("p g (t d) -> p g t d", d=2),
                    nv.rearrange("p g d t -> p g t d"),
                    Alu.subtract,
                )
                if post:
                    # y_post[k] = y_{b-1} + (k+1)*v_{b-1}
                    #           = rampneg[k]*nv_{b-1} + y_{b-1}   (ACT)
                    for gg in range(G):
                        for d in range(D):
                            nv1 = nv[:, gg, d, b - 1 : b]
                            y1 = y[:, gg, cb - 2 + d : cb - 1 + d]
                            dst = y[:, gg, cb + d : cf : 2]
                            nc.scalar.activation(
                                dst, ramp_t, Act.Identity, bias=y1, scale=nv1
                            )
                    nc.sync.dma_start(out=outv[ci, :, cs, :, 0:cb], in_=y[:, :, 0:cb])
                    nc.sync.dma_start(out=outv[ci, :, cs, :, cb:], in_=y[:, :, cb:])
                else:
                    nc.sync.dma_start(out=outv[ci, :, cs], in_=y)
    nc.compile()
    return nc


_NC_CACHE: dict = {}


def _get_nc(b_eff, mask_u8):
    key = (b_eff, mask_u8)
    if key not in _NC_CACHE:
        if b_eff > 96:
            # larger burn region: shrink buffering to fit SBUF
            _NC_CACHE[key] = _build(b_eff, mask_u8, io_bufs=4, wk_bufs=3)
        else:
            _NC_CACHE[key] = _build(b_eff, mask_u8)
    return _NC_CACHE[key]


def kernel(source, mask, A=None, B=None, C=None, burn_in_steps=64, **_):
    global last_results
    source = np.ascontiguousarray(np.asarray(source, dtype=np.float32))
    mask = np.asarray(mask, dtype=np.float32)
    assert source.shape == (N, T, D), source.shape
    assert mask.shape == (N, T, D), mask.shape

    b = int(burn_in_steps)
    b_eff = T if b <= 0 else min(b, T)
    post = T - b_eff
    cb = 2 * b_eff
    G, io_g = 4, 2
    GIO = io_g * G
    NBIG = ROWS_CORE // (NPART * GIO)

    mask_burn = mask[:, :b_eff, :]
    mask_u8 = bool(((mask_burn == 0.0) | (mask_burn == 1.0)).all())
    nc = _get_nc(b_eff, mask_u8)

    if mask_u8:
        # pack burn-region mask as uint8 in the kernel's chunk layout:
        # [NCORES*NBIG, NPART, GIO*cb] contiguous
        mu8 = np.ascontiguousarray(mask_burn, dtype=np.float32).astype(np.uint8)
        mu8 = mu8.reshape(NCORES, NBIG, NPART, GIO * cb)
    else:
        mask_f = np.ascontiguousarray(mask)

    if post:
        ramp = -np.broadcast_to(
            np.arange(1, post + 1, dtype=np.float32), (NPART, post)
        ).copy()

    in_maps = []
    for c in range(NCORES):
        m = {"src": source[c * ROWS_CORE : (c + 1) * ROWS_CORE]}
        if mask_u8:
            m["msku8"] = mu8[c]
        else:
            m["msk"] = mask_f[c * ROWS_CORE : (c + 1) * ROWS_CORE]
        if post:
            m["rampneg"] = ramp
        in_maps.append(m)

    res = run_bass_kernel_spmd(nc, in_maps, core_ids=list(range(NCORES)))
    last_results = res
    return np.concatenate([r["out"] for r in res.results], axis=0)


# revision 12
# speedup vs baseline: 1.0669x; 1.0669x over previous
"""Trainium2 Bass kernel for the Inertia model (nn_Net_55224689492388).

Math (exact restructuring of the reference scan; per (row n, channel d)):

  burn-in (t < b):
    app_t = (1 - mask_{t-1}) * mask_t        (mask_{-1} = 0)
    dx_t  = src_t - src_{t-1}                (src_{-1} = 0)
    v_t   = app_t * v_{t-1} + dx_t * (1 - app_t)
    y_t   = src_t + v_t
  post (t >= b): v stays constant (x_t - prev_x collapses to v_{t-1}), so
    y_t   = y_{b-1} + (t - b + 1) * v_{b-1}

Only v is sequential - a first-order linear recurrence computed with the
DVE TensorTensorScan instruction; everything else is bulk elementwise.
The kernel loads only the first b timesteps of src/mask (later steps
cannot affect the output) and writes the full output.

Implementation notes:
- Sign trick: nbt = (app - 1) * dx (one scalar_tensor_tensor op) makes
  the scan compute nv = -v, so y_burn = src - nv and the post phase uses
  a host-provided ramp of -(k+1): y_post = rampneg * nv_b1 + y_b1.
- Scan batching: zeroing the scan multiplier at each sequence's first
  element makes the scan self-initializing (v_0 = 0 * carry + nbt_0), so
  a single scan instruction covers every (group, channel) sequence of a
  chunk; intermediates are channel-deinterleaved (d-major) making it one
  contiguous stride-1 pass. The true app_0 enters through nbt_0, patched
  by a tiny one-column op.
- The binary mask ({0,1} from randint) travels as uint8, host-packed
  d-major so each partition's chunk slice is one contiguous >=512B DMA
  run; app is then a single is_lt compare. Falls back to f32 mask loads
  and the general (1-m_prev)*m_t arithmetic if the mask is non-binary.
- Engines: DVE does app/nbt/scan and most of y_burn; Pool does dx and
  the rest of y_burn; ACT does the post-phase extrapolation; all DMA on
  the SP HWDGE queue, output split burn/post for finer overlap.

Sharding: pure data parallel - 65536 rows split as 8192 rows x 8 cores,
no cross-core communication.
"""

import numpy as np

import concourse.bacc as bacc
import concourse.mybir as mybir
from concourse.bass_utils import run_bass_kernel_spmd
from concourse.tile import TileContext

N, T, D = 65536, 128, 2
NCORES = 8
NPART = 128
ROWS_CORE = N // NCORES  # 8192
G = 4  # row-groups per partition per compute chunk
IO_G = 2  # io tiles/DMAs cover IO_G*G groups
GIO = IO_G * G

F32 = mybir.dt.float32
U8 = mybir.dt.uint8
Alu = mybir.AluOpType
Act = mybir.ActivationFunctionType

# Stash of the most recent BassKernelResults (for test.py profiling).
last_results = None


def _pick_bufs(b, mask_u8):
    """Largest (io_bufs, wk_bufs) <= (9, 6) fitting the SBUF budget."""
    cb = 2 * b
    io_per = GIO * ((2 + cb) * 4 + (cb if mask_u8 else cb * 4) + 2 * T * 4)
    wk_per = G * cb * 4 * (4 if mask_u8 else 5)  # app,dx,nbt,nv (+omm)
    budget = 180 * 1024
    io_bufs, wk_bufs = 9, 6
    while io_bufs > 2 and io_bufs * io_per + wk_bufs * wk_per > budget:
        io_bufs -= 1
        if wk_bufs > 2:
            wk_bufs -= 1
    return io_bufs, wk_bufs


def _build(b, mask_u8=True):
    """Build the per-core Bass module for effective burn-in b (1..T)."""
    NCHUNK = ROWS_CORE // (NPART * G)
    NBIG = ROWS_CORE // (NPART * GIO)
    post = T - b
    cb = 2 * b  # burn-region columns (t-major, d-interleaved)
    cf = 2 * T  # full row columns
    io_bufs, wk_bufs = _pick_bufs(b, mask_u8)

    nc = bacc.Bacc("TRN2", target_bir_lowering=False, debug=False)
    src = nc.dram_tensor("src", [ROWS_CORE, T, D], F32, kind="ExternalInput")
    if mask_u8:
        # host-packed, d-major burn-region mask bytes
        msk = nc.dram_tensor(
            "msku8", [NBIG, NPART, IO_G, G, D, b], U8, kind="ExternalInput"
        )
    else:
        msk = nc.dram_tensor("msk", [ROWS_CORE, T, D], F32, kind="ExternalInput")
    out = nc.dram_tensor("out", [ROWS_CORE, T, D], F32, kind="ExternalOutput")
    if post:
        rampneg = nc.dram_tensor("rampneg", [NPART, post], F32, kind="ExternalInput")

    # row = ci*(128*GIO) + p*GIO + a*G + g : each partition holds GIO
    # consecutive rows, so the output DMA sees large contiguous runs.
    srcv = src[:].rearrange("(c p a g) t d -> c p a g (t d)", p=NPART, a=IO_G, g=G)
    outv = out[:].rearrange("(c p a g) t d -> c p a g (t d)", p=NPART, a=IO_G, g=G)
    if mask_u8:
        mskv = msk[:]
    else:
        mskv = msk[:].rearrange(
            "(c p a g) t d -> c p a g (t d)", p=NPART, a=IO_G, g=G
        )

    with TileContext(nc) as tc:
        with (
            tc.tile_pool(name="const", bufs=1) as cpool,
            tc.tile_pool(name="io", bufs=io_bufs) as iop,
            tc.tile_pool(name="wk", bufs=wk_bufs) as wkp,
        ):
            if post:
                ramp_t = cpool.tile([NPART, post], F32, name="ramp_t")
                nc.sync.dma_start(out=ramp_t, in_=rampneg[:])

            s_big = m_big = y_big = None
            for c in range(NCHUNK):
                ci, cs = divmod(c, IO_G)
                if cs == 0:
                    s_big = iop.tile([NPART, IO_G, G, 2 + cb], F32, name="s_ext")
                    if mask_u8:
                        m_big = iop.tile([NPART, IO_G, G, D, b], U8, name="m_t")
                    else:
                        m_big = iop.tile([NPART, IO_G, G, cb], F32, name="m_t")
                    y_big = iop.tile([NPART, IO_G, G, cf], F32, name="y")
                    nc.vector.memset(s_big[:, :, :, 0:2], 0.0)  # src_{-1} = 0
                    nc.sync.dma_start(
                        out=s_big[:, :, :, 2:], in_=srcv[ci, :, :, :, 0:cb]
                    )
                    if mask_u8:
                        nc.sync.dma_start(out=m_big, in_=mskv[ci])
                    else:
                        nc.sync.dma_start(out=m_big, in_=mskv[ci, :, :, :, 0:cb])
                s_ext = s_big[:, cs]
                m_t = m_big[:, cs]
                y = y_big[:, cs]

                # d-major intermediates: [p, g, d, t]
                app = wkp.tile([NPART, G, D, b], F32, name="app")
                dx = wkp.tile([NPART, G, D, b], F32, name="dx")
                nbt = wkp.tile([NPART, G, D, b], F32, name="nbt")
                nv = wkp.tile([NPART, G, D, b], F32, name="nv")

                s_hi4 = s_ext[:, :, 2:].rearrange("p g (t d) -> p g d t", d=D)
                s_lo4 = s_ext[:, :, 0:cb].rearrange("p g (t d) -> p g d t", d=D)

                if mask_u8:
                    # app_t = m_{t-1} < m_t for t>=1 (binary (1-m_prev)*m_t);
                    # col 0 stays 0 for the self-initializing scan.
                    nc.vector.memset(app[:, :, :, 0:1], 0.0)
                    if b > 1:
                        nc.vector.tensor_tensor(
                            app[:, :, :, 1:],
                            m_t[:, :, :, 0 : b - 1],
                            m_t[:, :, :, 1:],
                            Alu.is_lt,
                        )
                else:
                    omm = wkp.tile([NPART, G, D, 1 + b], F32, name="omm")
                    m4 = m_t.rearrange("p g (t d) -> p g d t", d=D)
                    nc.gpsimd.memset(omm[:, :, :, 0:1], 1.0)
                    nc.gpsimd.tensor_scalar(
                        omm[:, :, :, 1:], m4, -1.0, 1.0, Alu.mult, Alu.add
                    )
                    nc.gpsimd.tensor_tensor(app, omm[:, :, :, 0:b], m4, Alu.mult)

                # dx = src_t - src_{t-1} (Pool)
                nc.gpsimd.tensor_tensor(dx, s_hi4, s_lo4, Alu.subtract)
                # nbt = (app - 1) * dx = -dx*(1-app)
                nc.vector.scalar_tensor_tensor(
                    nbt, app, 1.0, dx, Alu.subtract, Alu.mult
                )
                if mask_u8:
                    # true nbt_0 = (m_0 - 1) * src_0 (app_0 = m_0, v_{-1} = 0)
                    nc.vector.scalar_tensor_tensor(
                        nbt[:, :, :, 0:1],
                        m_t[:, :, :, 0:1],
                        1.0,
                        s_hi4[:, :, :, 0:1],
                        Alu.subtract,
                        Alu.mult,
                    )
                else:
                    # zero each sequence's first multiplier after nbt read it
                    nc.vector.memset(app[:, :, :, 0:1], 0.0)

                # single scan across all (g, d) sequences: nv = -v
                nc.vector.tensor_tensor_scan(
                    nv.rearrange("p g d t -> p (g d t)"),
                    app.rearrange("p g d t -> p (g d t)"),
                    nbt.rearrange("p g d t -> p (g d t)"),
                    0.0,
                    Alu.mult,
                    Alu.add,
                )

                # y_burn = src + v = src - nv (3 groups DVE, 1 group Pool)
                y4 = y[:, :, 0:cb].rearrange("p g (t d) -> p g t d", d=D)
                sh4 = s_ext[:, :, 2:].rearrange("p g (t d) -> p g t d", d=D)
                nv4 = nv.rearrange("p g d t -> p g t d")
                ky = G - 1
                nc.vector.tensor_tensor(
                    y4[:, 0:ky], sh4[:, 0:ky], nv4[:, 0:ky], Alu.subtract
                )
                nc.gpsimd.tensor_tensor(
                    y4[:, ky:], sh4[:, ky:], nv4[:, ky:], Alu.subtract
                )

                if post:
                    # y_post[k] = y_{b-1} + (k+1)*v_{b-1}
                    #           = rampneg[k]*nv_{b-1} + y_{b-1}   (ACT)
                    for gg in range(G):
                        for d in range(D):
                            nv1 = nv[:, gg, d, b - 1 : b]
                            y1 = y[:, gg, cb - 2 + d : cb - 1 + d]
                            dst = y[:, gg, cb + d : cf : 2]
                            nc.scalar.activation(
                                dst, ramp_t, Act.Identity, bias=y1, scale=nv1
                            )
                    nc.sync.dma_start(out=outv[ci, :, cs, :, 0:cb], in_=y[:, :, 0:cb])
                    nc.sync.dma_start(out=outv[ci, :, cs, :, cb:], in_=y[:, :, cb:])
                else:
                    nc.sync.dma_start(out=outv[ci, :, cs], in_=y)
    nc.compile()
    return nc


_NC_CACHE: dict = {}


def kernel(source, mask, A=None, B=None, C=None, burn_in_steps=64, **_):
    global last_results
    source = np.ascontiguousarray(np.asarray(source, dtype=np.float32))
    mask = np.asarray(mask, dtype=np.float32)
    assert source.shape == (N, T, D), source.shape
    assert mask.shape == (N, T, D), mask.shape

    b = int(burn_in_steps)
    b_eff = T if b <= 0 else min(b, T)
    post = T - b_eff
    NBIG = ROWS_CORE // (NPART * GIO)

    mask_burn = mask[:, :b_eff, :]
    mask_u8 = bool(((mask_burn == 0.0) | (mask_burn == 1.0)).all())

    key = (b_eff, mask_u8)
    if key not in _NC_CACHE:
        _NC_CACHE[key] = _build(b_eff, mask_u8)
    nc = _NC_CACHE[key]

    if mask_u8:
        # pack burn-region mask as d-major uint8 in the kernel's chunk
        # layout: [NCORES*NBIG, NPART, GIO, D, b] contiguous
        mu8 = np.ascontiguousarray(mask_burn.transpose(0, 2, 1)).astype(np.uint8)
        mu8 = mu8.reshape(NCORES, NBIG, NPART, GIO, D, b_eff)
    else:
        mask_f = np.ascontiguousarray(mask)

    if post:
        ramp = -np.broadcast_to(
            np.arange(1, post + 1, dtype=np.float32), (NPART, post)
        ).copy()

    in_maps = []
    for c in range(NCORES):
        m = {"src": source[c * ROWS_CORE : (c + 1) * ROWS_CORE]}
        if mask_u8:
            m["msku8"] = mu8[c]
        else:
            m["msk"] = mask_f[c * ROWS_CORE : (c + 1) * ROWS_CORE]
        if post:
            m["rampneg"] = ramp
        in_maps.append(m)

    res = run_bass_kernel_spmd(nc, in_maps, core_ids=list(range(NCORES)))
    last_results = res
    return np.concatenate([r["out"] for r in res.results], axis=0)


# revision 13
# speedup vs baseline: 1.0852x; 1.0172x over previous
"""Trainium2 Bass kernel for the Inertia model (nn_Net_55224689492388).

Math (exact restructuring of the reference scan; per (row n, channel d)):

  burn-in (t < b):
    app_t = (1 - mask_{t-1}) * mask_t        (mask_{-1} = 0)
    dx_t  = src_t - src_{t-1}                (src_{-1} = 0)
    v_t   = app_t * v_{t-1} + dx_t * (1 - app_t)
    y_t   = src_t + v_t
  post (t >= b): v stays constant (x_t - prev_x collapses to v_{t-1}), so
    y_t   = y_{b-1} + (t - b + 1) * v_{b-1}

Only v is sequential - a first-order linear recurrence computed with the
DVE TensorTensorScan instruction; everything else is bulk elementwise.
The kernel loads only the first b timesteps of src/mask (later steps
cannot affect the output) and writes the full output.

Implementation notes:
- Sign trick: nbt = (app - 1) * dx (one scalar_tensor_tensor op) makes
  the scan compute nv = -v, so y_burn = src - nv and the post phase uses
  a host-provided ramp of -(k+1): y_post = rampneg * nv_b1 + y_b1.
- Scan batching: zeroing the scan multiplier at each sequence's first
  element makes the scan self-initializing (v_0 = 0 * carry + nbt_0), so
  a single scan instruction covers every (group, channel) sequence of a
  chunk; intermediates are channel-deinterleaved (d-major) making it one
  contiguous stride-1 pass. The true app_0 enters through nbt_0, patched
  by a tiny one-column op.
- The binary mask ({0,1} from randint) travels as uint8, host-packed
  d-major so each partition's chunk slice is one contiguous >=512B DMA
  run; app is then a single is_lt compare. Falls back to f32 mask loads
  and the general (1-m_prev)*m_t arithmetic if the mask is non-binary.
- Engines: DVE does app/nbt/scan and most of y_burn; Pool does dx and
  the rest of y_burn; ACT does the post-phase extrapolation; all DMA on
  the SP HWDGE queue, output split burn/post for finer overlap.

Sharding: pure data parallel - 65536 rows split as 8192 rows x 8 cores,
no cross-core communication.
"""

import numpy as np

import concourse.bacc as bacc
import concourse.mybir as mybir
from concourse.bass_utils import run_bass_kernel_spmd
from concourse.tile import TileContext

N, T, D = 65536, 128, 2
NCORES = 8
NPART = 128
ROWS_CORE = N // NCORES  # 8192
G = 4  # row-groups per partition per compute chunk
IO_G = 2  # io tiles/DMAs cover IO_G*G groups
GIO = IO_G * G

F32 = mybir.dt.float32
U8 = mybir.dt.uint8
Alu = mybir.AluOpType
Act = mybir.ActivationFunctionType

# Stash of the most recent BassKernelResults (for test.py profiling).
last_results = None


def _pick_bufs(b, mask_u8):
    """Largest (io_bufs, wk_bufs) <= (9, 6) fitting the SBUF budget."""
    cb = 2 * b
    io_per = GIO * ((2 + cb) * 4 + (cb if mask_u8 else cb * 4) + 2 * T * 4)
    wk_per = G * cb * 4 * (4 if mask_u8 else 5)  # app,dx,nbt,nv (+omm)
    budget = 180 * 1024
    io_bufs, wk_bufs = 9, 6
    while io_bufs > 2 and io_bufs * io_per + wk_bufs * wk_per > budget:
        io_bufs -= 1
        if wk_bufs > 2:
            wk_bufs -= 1
    return io_bufs, wk_bufs


def _build(b, mask_u8=True):
    """Build the per-core Bass module for effective burn-in b (1..T)."""
    NCHUNK = ROWS_CORE // (NPART * G)
    NBIG = ROWS_CORE // (NPART * GIO)
    post = T - b
    cb = 2 * b  # burn-region columns (t-major, d-interleaved)
    cf = 2 * T  # full row columns
    io_bufs, wk_bufs = _pick_bufs(b, mask_u8)

    nc = bacc.Bacc("TRN2", target_bir_lowering=False, debug=False)
    src = nc.dram_tensor("src", [ROWS_CORE, T, D], F32, kind="ExternalInput")
    if mask_u8:
        # host-packed, d-major burn-region mask bytes
        msk = nc.dram_tensor(
            "msku8", [NBIG, NPART, IO_G, G, D, b], U8, kind="ExternalInput"
        )
    else:
        msk = nc.dram_tensor("msk", [ROWS_CORE, T, D], F32, kind="ExternalInput")
    out = nc.dram_tensor("out", [ROWS_CORE, T, D], F32, kind="ExternalOutput")
    if post:
        rampneg = nc.dram_tensor("rampneg", [NPART, post], F32, kind="ExternalInput")

    # row = ci*(128*GIO) + p*GIO + a*G + g : each partition holds GIO
    # consecutive rows, so the output DMA sees large contiguous runs.
    srcv = src[:].rearrange("(c p a g) t d -> c p a g (t d)", p=NPART, a=IO_G, g=G)
    outv = out[:].rearrange("(c p a g) t d -> c p a g (t d)", p=NPART, a=IO_G, g=G)
    if mask_u8:
        mskv = msk[:]
    else:
        mskv = msk[:].rearrange(
            "(c p a g) t d -> c p a g (t d)", p=NPART, a=IO_G, g=G
        )

    with TileContext(nc) as tc:
        with (
            tc.tile_pool(name="const", bufs=1) as cpool,
            tc.tile_pool(name="io", bufs=io_bufs) as iop,
            tc.tile_pool(name="wk", bufs=wk_bufs) as wkp,
        ):
            if post:
                ramp_t = cpool.tile([NPART, post], F32, name="ramp_t")
                nc.sync.dma_start(out=ramp_t, in_=rampneg[:])

            s_big = m_big = y_big = None
            for c in range(NCHUNK):
                ci, cs = divmod(c, IO_G)
                if cs == 0:
                    s_big = iop.tile([NPART, IO_G, G, 2 + cb], F32, name="s_ext")
                    if mask_u8:
                        m_big = iop.tile([NPART, IO_G, G, D, b], U8, name="m_t")
                    else:
                        m_big = iop.tile([NPART, IO_G, G, cb], F32, name="m_t")
                    y_big = iop.tile([NPART, IO_G, G, cf], F32, name="y")
                    nc.sync.dma_start(
                        out=s_big[:, :, :, 2:], in_=srcv[ci, :, :, :, 0:cb]
                    )
                    if mask_u8:
                        nc.sync.dma_start(out=m_big, in_=mskv[ci])
                    else:
                        nc.sync.dma_start(out=m_big, in_=mskv[ci, :, :, :, 0:cb])
                    # src_{-1} = 0; emitted after the DMAs: the later priority
                    # schedules measurably better (model: 51.2 -> 50.4 us)
                    nc.vector.memset(s_big[:, :, :, 0:2], 0.0)
                s_ext = s_big[:, cs]
                m_t = m_big[:, cs]
                y = y_big[:, cs]

                # d-major intermediates: [p, g, d, t]
                app = wkp.tile([NPART, G, D, b], F32, name="app")
                dx = wkp.tile([NPART, G, D, b], F32, name="dx")
                nbt = wkp.tile([NPART, G, D, b], F32, name="nbt")
                nv = wkp.tile([NPART, G, D, b], F32, name="nv")

                s_hi4 = s_ext[:, :, 2:].rearrange("p g (t d) -> p g d t", d=D)
                s_lo4 = s_ext[:, :, 0:cb].rearrange("p g (t d) -> p g d t", d=D)

                if mask_u8:
                    # app_t = m_{t-1} < m_t for t>=1 (binary (1-m_prev)*m_t);
                    # col 0 stays 0 for the self-initializing scan.
                    nc.vector.memset(app[:, :, :, 0:1], 0.0)
                    if b > 1:
                        nc.vector.tensor_tensor(
                            app[:, :, :, 1:],
                            m_t[:, :, :, 0 : b - 1],
                            m_t[:, :, :, 1:],
                            Alu.is_lt,
                        )
                else:
                    omm = wkp.tile([NPART, G, D, 1 + b], F32, name="omm")
                    m4 = m_t.rearrange("p g (t d) -> p g d t", d=D)
                    nc.gpsimd.memset(omm[:, :, :, 0:1], 1.0)
                    nc.gpsimd.tensor_scalar(
                        omm[:, :, :, 1:], m4, -1.0, 1.0, Alu.mult, Alu.add
                    )
                    nc.gpsimd.tensor_tensor(app, omm[:, :, :, 0:b], m4, Alu.mult)

                # dx = src_t - src_{t-1} (Pool)
                nc.gpsimd.tensor_tensor(dx, s_hi4, s_lo4, Alu.subtract)
                # nbt = (app - 1) * dx = -dx*(1-app)
                nc.vector.scalar_tensor_tensor(
                    nbt, app, 1.0, dx, Alu.subtract, Alu.mult
                )
                if mask_u8:
                    # true nbt_0 = (m_0 - 1) * src_0 (app_0 = m_0, v_{-1} = 0)
                    nc.vector.scalar_tensor_tensor(
                        nbt[:, :, :, 0:1],
                        m_t[:, :, :, 0:1],
                        1.0,
                        s_hi4[:, :, :, 0:1],
                        Alu.subtract,
                        Alu.mult,
                    )
                else:
                    # zero each sequence's first multiplier after nbt read it
                    nc.vector.memset(app[:, :, :, 0:1], 0.0)

                # single scan across all (g, d) sequences: nv = -v
                nc.vector.tensor_tensor_scan(
                    nv.rearrange("p g d t -> p (g d t)"),
                    app.rearrange("p g d t -> p (g d t)"),
                    nbt.rearrange("p g d t -> p (g d t)"),
                    0.0,
                    Alu.mult,
                    Alu.add,
                )

                # y_burn = src + v = src - nv (3 groups DVE, 1 group Pool)
                y4 = y[:, :, 0:cb].rearrange("p g (t d) -> p g t d", d=D)
                sh4 = s_ext[:, :, 2:].rearrange("p g (t d) -> p g t d", d=D)
                nv4 = nv.rearrange("p g d t -> p g t d")
                ky = G - 1
                nc.vector.tensor_tensor(
                    y4[:, 0:ky], sh4[:, 0:ky], nv4[:, 0:ky], Alu.subtract
                )
                nc.gpsimd.tensor_tensor(
                    y4[:, ky:], sh4[:, ky:], nv4[:, ky:], Alu.subtract
                )

                if post:
                    # y_post[k] = y_{b-1} + (k+1)*v_{b-1}
                    #           = rampneg[k]*nv_{b-1} + y_{b-1}   (ACT)
                    for gg in range(G):
                        for d in range(D):
                            nv1 = nv[:, gg, d, b - 1 : b]
                            y1 = y[:, gg, cb - 2 + d : cb - 1 + d]
                            dst = y[:, gg, cb + d : cf : 2]
                            nc.scalar.activation(
                                dst, ramp_t, Act.Identity, bias=y1, scale=nv1
                            )
                    nc.sync.dma_start(out=outv[ci, :, cs, :, 0:cb], in_=y[:, :, 0:cb])
                    nc.sync.dma_start(out=outv[ci, :, cs, :, cb:], in_=y[:, :, cb:])
                else:
                    nc.sync.dma_start(out=outv[ci, :, cs], in_=y)
    nc.compile()
    return nc


_NC_CACHE: dict = {}


def kernel(source, mask, A=None, B=None, C=None, burn_in_steps=64, **_):
    global last_results
    source = np.ascontiguousarray(np.asarray(source, dtype=np.float32))
    mask = np.asarray(mask, dtype=np.float32)
    assert source.shape == (N, T, D), source.shape
    assert mask.shape == (N, T, D), mask.shape

    b = int(burn_in_steps)
    b_eff = T if b <= 0 else min(b, T)
    post = T - b_eff
    NBIG = ROWS_CORE // (NPART * GIO)

    mask_burn = mask[:, :b_eff, :]
    mask_u8 = bool(((mask_burn == 0.0) | (mask_burn == 1.0)).all())

    key = (b_eff, mask_u8)
    if key not in _NC_CACHE:
        _NC_CACHE[key] = _build(b_eff, mask_u8)
    nc = _NC_CACHE[key]

    if mask_u8:
        # pack burn-region mask as d-major uint8 in the kernel's chunk
        # layout: [NCORES*NBIG, NPART, GIO, D, b] contiguous
        mu8 = np.ascontiguousarray(mask_burn.transpose(0, 2, 1)).astype(np.uint8)
        mu8 = mu8.reshape(NCORES, NBIG, NPART, GIO, D, b_eff)
    else:
        mask_f = np.ascontiguousarray(mask)

    if post:
        ramp = -np.broadcast_to(
            np.arange(1, post + 1, dtype=np.float32), (NPART, post)
        ).copy()

    in_maps = []
    for c in range(NCORES):
        m = {"src": source[c * ROWS_CORE : (c + 1) * ROWS_CORE]}
        if mask_u8:
            m["msku8"] = mu8[c]
        else:
            m["msk"] = mask_f[c * ROWS_CORE : (c + 1) * ROWS_CORE]
        if post:
            m["rampneg"] = ramp
        in_maps.append(m)

    res = run_bass_kernel_spmd(nc, in_maps, core_ids=list(range(NCORES)))
    last_results = res
    return np.concatenate([r["out"] for r in res.results], axis=0)


# revision 17
# speedup vs baseline: 1.0986x; 1.0124x over previous
"""Trainium2 Bass kernel for the Inertia model (nn_Net_55224689492388).

Math (exact restructuring of the reference scan; per (row n, channel d)):

  burn-in (t < b):
    app_t = (1 - mask_{t-1}) * mask_t        (mask_{-1} = 0)
    dx_t  = src_t - src_{t-1}                (src_{-1} = 0)
    v_t   = app_t * v_{t-1} + dx_t * (1 - app_t)
    y_t   = src_t + v_t
  post (t >= b): v stays constant (x_t - prev_x collapses to v_{t-1}), so
    y_t   = y_{b-1} + (t - b + 1) * v_{b-1}

Only v is sequential - a first-order linear recurrence computed with the
DVE TensorTensorScan instruction; everything else is bulk elementwise.
The kernel loads only the first b timesteps of src/mask (later steps
cannot affect the output) and writes the full output.

Implementation notes:
- Sign trick: nbt = (app - 1) * dx (one scalar_tensor_tensor op) makes
  the scan compute nv = -v, so y_burn = src - nv and the post phase uses
  a host-provided ramp of -(k+1): y_post = rampneg * nv_b1 + y_b1.
- Scan batching: zeroing the scan multiplier at each sequence's first
  element makes the scan self-initializing (v_0 = 0 * carry + nbt_0), so
  a single scan instruction covers every (group, channel) sequence of a
  chunk; intermediates are channel-deinterleaved (d-major) making it one
  contiguous stride-1 pass. The true app_0 enters through nbt_0, patched
  by a tiny one-column op.
- The binary mask ({0,1} from randint) travels as uint8, host-packed
  d-major so each partition's chunk slice is one contiguous >=512B DMA
  run; app is then a single is_lt compare. Falls back to f32 mask loads
  and the general (1-m_prev)*m_t arithmetic if the mask is non-binary.
- Engines: DVE does app/nbt/scan and most of y_burn; Pool does dx and
  the rest of y_burn; ACT does the post-phase extrapolation; all DMA on
  the SP HWDGE queue, output split burn/post for finer overlap.

Sharding: pure data parallel - 65536 rows split as 8192 rows x 8 cores,
no cross-core communication.
"""

import numpy as np

import concourse.bacc as bacc
import concourse.mybir as mybir
from concourse.bass_utils import run_bass_kernel_spmd
from concourse.tile import TileContext

N, T, D = 65536, 128, 2
NCORES = 8
NPART = 128
ROWS_CORE = N // NCORES  # 8192
G = 4  # row-groups per partition per compute chunk
IO_G = 2  # io tiles/DMAs cover IO_G*G groups
GIO = IO_G * G

F32 = mybir.dt.float32
U8 = mybir.dt.uint8
Alu = mybir.AluOpType
Act = mybir.ActivationFunctionType

# Stash of the most recent BassKernelResults (for test.py profiling).
last_results = None


def _pick_bufs(b, mask_u8):
    """Largest (io_bufs, wk_bufs) <= (9, 6) fitting the SBUF budget."""
    cb = 2 * b
    io_per = GIO * ((2 + cb) * 4 + (cb if mask_u8 else cb * 4) + 2 * T * 4)
    wk_per = G * cb * 4 * (4 if mask_u8 else 5)  # app,dx,nbt,nv (+omm)
    budget = 180 * 1024
    io_bufs, wk_bufs = 9, 6
    while io_bufs > 2 and io_bufs * io_per + wk_bufs * wk_per > budget:
        io_bufs -= 1
        if wk_bufs > 2:
            wk_bufs -= 1
    return io_bufs, wk_bufs


def _build(b, mask_u8=True):
    """Build the per-core Bass module for effective burn-in b (1..T)."""
    NCHUNK = ROWS_CORE // (NPART * G)
    NBIG = ROWS_CORE // (NPART * GIO)
    post = T - b
    cb = 2 * b  # burn-region columns (t-major, d-interleaved)
    cf = 2 * T  # full row columns
    io_bufs, wk_bufs = _pick_bufs(b, mask_u8)

    nc = bacc.Bacc("TRN2", target_bir_lowering=False, debug=False)
    src = nc.dram_tensor("src", [ROWS_CORE, T, D], F32, kind="ExternalInput")
    if mask_u8:
        # host-packed, d-major burn-region mask bytes
        msk = nc.dram_tensor(
            "msku8", [NBIG, NPART, IO_G, G, D, b], U8, kind="ExternalInput"
        )
    else:
        msk = nc.dram_tensor("msk", [ROWS_CORE, T, D], F32, kind="ExternalInput")
    out = nc.dram_tensor("out", [ROWS_CORE, T, D], F32, kind="ExternalOutput")
    if post:
        rampneg = nc.dram_tensor("rampneg", [NPART, post], F32, kind="ExternalInput")

    # row = ci*(128*GIO) + p*GIO + a*G + g : each partition holds GIO
    # consecutive rows, so the output DMA sees large contiguous runs.
    srcv = src[:].rearrange("(c p a g) t d -> c p a g (t d)", p=NPART, a=IO_G, g=G)
    outv = out[:].rearrange("(c p a g) t d -> c p a g (t d)", p=NPART, a=IO_G, g=G)
    if mask_u8:
        mskv = msk[:]
    else:
        mskv = msk[:].rearrange(
            "(c p a g) t d -> c p a g (t d)", p=NPART, a=IO_G, g=G
        )

    with TileContext(nc) as tc:
        with (
            tc.tile_pool(name="const", bufs=1) as cpool,
            tc.tile_pool(name="io", bufs=io_bufs) as iop,
            tc.tile_pool(name="wk", bufs=wk_bufs) as wkp,
        ):
            if post:
                ramp_t = cpool.tile([NPART, post], F32, name="ramp_t")
                nc.sync.dma_start(out=ramp_t, in_=rampneg[:])

            s_big = m_big = y_big = None
            for c in range(NCHUNK):
                ci, cs = divmod(c, IO_G)
                if cs == 0:
                    s_big = iop.tile([NPART, IO_G, G, 2 + cb], F32, name="s_ext")
                    if mask_u8:
                        m_big = iop.tile([NPART, IO_G, G, D, b], U8, name="m_t")
                    else:
                        m_big = iop.tile([NPART, IO_G, G, cb], F32, name="m_t")
                    y_big = iop.tile([NPART, IO_G, G, cf], F32, name="y")
                    if mask_u8:
                        # per-sub-chunk src DMAs: finer-grained availability
                        for j in range(IO_G):
                            nc.sync.dma_start(
                                out=s_big[:, j : j + 1, :, 2:],
                                in_=srcv[ci, :, j : j + 1, :, 0:cb],
                            )
                        nc.sync.dma_start(out=m_big, in_=mskv[ci])
                        # no lead-column memset: in the u8 path cols 0:2 are
                        # never read (dx/nbt computed on t>=1; t=0 patched)
                    else:
                        nc.sync.dma_start(
                            out=s_big[:, :, :, 2:], in_=srcv[ci, :, :, :, 0:cb]
                        )
                        nc.sync.dma_start(out=m_big, in_=mskv[ci, :, :, :, 0:cb])
                        nc.vector.memset(s_big[:, :, :, 0:2], 0.0)  # src_{-1}=0
                s_ext = s_big[:, cs]
                m_t = m_big[:, cs]
                y = y_big[:, cs]

                # d-major intermediates: [p, g, d, t]
                app = wkp.tile([NPART, G, D, b], F32, name="app")
                dx = wkp.tile([NPART, G, D, b], F32, name="dx")
                nbt = wkp.tile([NPART, G, D, b], F32, name="nbt")
                nv = wkp.tile([NPART, G, D, b], F32, name="nv")

                s_hi4 = s_ext[:, :, 2:].rearrange("p g (t d) -> p g d t", d=D)
                s_lo4 = s_ext[:, :, 0:cb].rearrange("p g (t d) -> p g d t", d=D)

                if mask_u8:
                    # app_t = m_{t-1} < m_t for t>=1 (binary (1-m_prev)*m_t);
                    # col 0 stays 0 for the self-initializing scan.
                    nc.vector.memset(app[:, :, :, 0:1], 0.0)
                    if b > 1:
                        nc.vector.tensor_tensor(
                            app[:, :, :, 1:],
                            m_t[:, :, :, 0 : b - 1],
                            m_t[:, :, :, 1:],
                            Alu.is_lt,
                        )
                    # dx/nbt only for t>=1 (t=0 handled by the nbt_0 patch;
                    # the unread lead cols of s_ext then need no memset)
                    if b > 1:
                        nc.gpsimd.tensor_tensor(
                            dx[:, :, :, 1:],
                            s_hi4[:, :, :, 1:],
                            s_lo4[:, :, :, 1:],
                            Alu.subtract,
                        )
                        nc.vector.scalar_tensor_tensor(
                            nbt[:, :, :, 1:],
                            app[:, :, :, 1:],
                            1.0,
                            dx[:, :, :, 1:],
                            Alu.subtract,
                            Alu.mult,
                        )
                    # true nbt_0 = (m_0 - 1) * src_0 (app_0 = m_0, v_{-1} = 0)
                    nc.vector.scalar_tensor_tensor(
                        nbt[:, :, :, 0:1],
                        m_t[:, :, :, 0:1],
                        1.0,
                        s_hi4[:, :, :, 0:1],
                        Alu.subtract,
                        Alu.mult,
                    )
                else:
                    omm = wkp.tile([NPART, G, D, 1 + b], F32, name="omm")
                    m4 = m_t.rearrange("p g (t d) -> p g d t", d=D)
                    nc.gpsimd.memset(omm[:, :, :, 0:1], 1.0)
                    nc.gpsimd.tensor_scalar(
                        omm[:, :, :, 1:], m4, -1.0, 1.0, Alu.mult, Alu.add
                    )
                    nc.gpsimd.tensor_tensor(app, omm[:, :, :, 0:b], m4, Alu.mult)
                    # dx = src_t - src_{t-1} (Pool)
                    nc.gpsimd.tensor_tensor(dx, s_hi4, s_lo4, Alu.subtract)
                    # nbt = (app - 1) * dx = -dx*(1-app)
                    nc.vector.scalar_tensor_tensor(
                        nbt, app, 1.0, dx, Alu.subtract, Alu.mult
                    )
                    # zero each sequence's first multiplier after nbt read it
                    nc.vector.memset(app[:, :, :, 0:1], 0.0)

                # single scan across all (g, d) sequences: nv = -v
                nc.vector.tensor_tensor_scan(
                    nv.rearrange("p g d t -> p (g d t)"),
                    app.rearrange("p g d t -> p (g d t)"),
                    nbt.rearrange("p g d t -> p (g d t)"),
                    0.0,
                    Alu.mult,
                    Alu.add,
                )

                # y_burn = src + v = src - nv (3 groups DVE, 1 group Pool).
                # The t=b-1 column goes first as a tiny separate op so the
                # ACT post ops (which only need y_{b-1}) unblock early.
                y4 = y[:, :, 0:cb].rearrange("p g (t d) -> p g t d", d=D)
                sh4 = s_ext[:, :, 2:].rearrange("p g (t d) -> p g t d", d=D)
                nv4 = nv.rearrange("p g d t -> p g t d")
                ky = G - 1
                tlo = b - 1 if post else b
                if post:
                    nc.vector.tensor_tensor(
                        y4[:, :, b - 1 :],
                        sh4[:, :, b - 1 :],
                        nv4[:, :, b - 1 :],
                        Alu.subtract,
                    )
                nc.vector.tensor_tensor(
                    y4[:, 0:ky, 0:tlo], sh4[:, 0:ky, 0:tlo],
                    nv4[:, 0:ky, 0:tlo], Alu.subtract
                )
                nc.gpsimd.tensor_tensor(
                    y4[:, ky:, 0:tlo], sh4[:, ky:, 0:tlo],
                    nv4[:, ky:, 0:tlo], Alu.subtract
                )

                if post:
                    # y_post[k] = y_{b-1} + (k+1)*v_{b-1}
                    #           = rampneg[k]*nv_{b-1} + y_{b-1}   (ACT)
                    for gg in range(G):
                        for d in range(D):
                            nv1 = nv[:, gg, d, b - 1 : b]
                            y1 = y[:, gg, cb - 2 + d : cb - 1 + d]
                            dst = y[:, gg, cb + d : cf : 2]
                            nc.scalar.activation(
                                dst, ramp_t, Act.Identity, bias=y1, scale=nv1
                            )
                    nc.sync.dma_start(out=outv[ci, :, cs, :, 0:cb], in_=y[:, :, 0:cb])
                    nc.sync.dma_start(out=outv[ci, :, cs, :, cb:], in_=y[:, :, cb:])
                else:
                    nc.sync.dma_start(out=outv[ci, :, cs], in_=y)
    nc.compile()
    return nc


_NC_CACHE: dict = {}


def kernel(source, mask, A=None, B=None, C=None, burn_in_steps=64, **_):
    global last_results
    source = np.ascontiguousarray(np.asarray(source, dtype=np.float32))
    mask = np.asarray(mask, dtype=np.float32)
    assert source.shape == (N, T, D), source.shape
    assert mask.shape == (N, T, D), mask.shape

    b = int(burn_in_steps)
    b_eff = T if b <= 0 else min(b, T)
    post = T - b_eff
    NBIG = ROWS_CORE // (NPART * GIO)

    mask_burn = mask[:, :b_eff, :]
    mask_u8 = bool(((mask_burn == 0.0) | (mask_burn == 1.0)).all())

    key = (b_eff, mask_u8)
    if key not in _NC_CACHE:
        _NC_CACHE[key] = _build(b_eff, mask_u8)
    nc = _NC_CACHE[key]

    if mask_u8:
        # pack burn-region mask as d-major uint8 in the kernel's chunk
        # layout: [NCORES*NBIG, NPART, GIO, D, b] contiguous
        mu8 = np.ascontiguousarray(mask_burn.transpose(0, 2, 1)).astype(np.uint8)
        mu8 = mu8.reshape(NCORES, NBIG, NPART, GIO, D, b_eff)
    else:
        mask_f = np.ascontiguousarray(mask)

    if post:
        ramp = -np.broadcast_to(
            np.arange(1, post + 1, dtype=np.float32), (NPART, post)
        ).copy()

    in_maps = []
    for c in range(NCORES):
        m = {"src": source[c * ROWS_CORE : (c + 1) * ROWS_CORE]}
        if mask_u8:
            m["msku8"] = mu8[c]
        else:
            m["msk"] = mask_f[c * ROWS_CORE : (c + 1) * ROWS_CORE]
        if post:
            m["rampneg"] = ramp
        in_maps.append(m)

    res = run_bass_kernel_spmd(nc, in_maps, core_ids=list(range(NCORES)))
    last_results = res
    return np.concatenate([r["out"] for r in res.results], axis=0)


# revision 18
# speedup vs baseline: 1.1046x; 1.0054x over previous
"""Trainium2 Bass kernel for the Inertia model (nn_Net_55224689492388).

Math (exact restructuring of the reference scan; per (row n, channel d)):

  burn-in (t < b):
    app_t = (1 - mask_{t-1}) * mask_t        (mask_{-1} = 0)
    dx_t  = src_t - src_{t-1}                (src_{-1} = 0)
    v_t   = app_t * v_{t-1} + dx_t * (1 - app_t)
    y_t   = src_t + v_t
  post (t >= b): v stays constant (x_t - prev_x collapses to v_{t-1}), so
    y_t   = y_{b-1} + (t - b + 1) * v_{b-1}

Only v is sequential - a first-order linear recurrence computed with the
DVE TensorTensorScan instruction; everything else is bulk elementwise.
The kernel loads only the first b timesteps of src/mask (later steps
cannot affect the output) and writes the full output.

Implementation notes:
- Sign trick: nbt = (app - 1) * dx (one scalar_tensor_tensor op) makes
  the scan compute nv = -v, so y_burn = src - nv and the post phase uses
  a host-provided ramp of -(k+1): y_post = rampneg * nv_b1 + y_b1.
- Scan batching: zeroing the scan multiplier at each sequence's first
  element makes the scan self-initializing (v_0 = 0 * carry + nbt_0), so
  a single scan instruction covers every (group, channel) sequence of a
  chunk; intermediates are channel-deinterleaved (d-major) making it one
  contiguous stride-1 pass. The true app_0 enters through nbt_0, patched
  by a tiny one-column op.
- The binary mask ({0,1} from randint) travels as uint8, host-packed
  d-major so each partition's chunk slice is one contiguous >=512B DMA
  run; app is then a single is_lt compare. Falls back to f32 mask loads
  and the general (1-m_prev)*m_t arithmetic if the mask is non-binary.
- Engines: DVE does app/nbt/scan and most of y_burn; Pool does dx and
  the rest of y_burn; ACT does the post-phase extrapolation; all DMA on
  the SP HWDGE queue, output split burn/post for finer overlap.

Sharding: pure data parallel - 65536 rows split as 8192 rows x 8 cores,
no cross-core communication.
"""

import numpy as np

import concourse.bacc as bacc
import concourse.mybir as mybir
from concourse.bass_utils import run_bass_kernel_spmd
from concourse.tile import TileContext

N, T, D = 65536, 128, 2
NCORES = 8
NPART = 128
ROWS_CORE = N // NCORES  # 8192
G = 4  # row-groups per partition per compute chunk
IO_G = 2  # io tiles/DMAs cover IO_G*G groups
GIO = IO_G * G

F32 = mybir.dt.float32
U8 = mybir.dt.uint8
Alu = mybir.AluOpType
Act = mybir.ActivationFunctionType

# Stash of the most recent BassKernelResults (for test.py profiling).
last_results = None


def _pick_bufs(b, mask_u8):
    """Largest (io_bufs, wk_bufs) <= (9, 6) fitting the SBUF budget."""
    cb = 2 * b
    io_per = GIO * ((2 + cb) * 4 + (cb if mask_u8 else cb * 4) + 2 * T * 4)
    wk_per = G * cb * 4 * (4 if mask_u8 else 5)  # app,dx,nbt,nv (+omm)
    budget = 180 * 1024
    io_bufs, wk_bufs = 9, 6
    while io_bufs > 2 and io_bufs * io_per + wk_bufs * wk_per > budget:
        io_bufs -= 1
        if wk_bufs > 2:
            wk_bufs -= 1
    return io_bufs, wk_bufs


def _build(b, mask_u8=True):
    """Build the per-core Bass module for effective burn-in b (1..T)."""
    NCHUNK = ROWS_CORE // (NPART * G)
    NBIG = ROWS_CORE // (NPART * GIO)
    post = T - b
    cb = 2 * b  # burn-region columns (t-major, d-interleaved)
    cf = 2 * T  # full row columns
    io_bufs, wk_bufs = _pick_bufs(b, mask_u8)

    nc = bacc.Bacc("TRN2", target_bir_lowering=False, debug=False)
    src = nc.dram_tensor("src", [ROWS_CORE, T, D], F32, kind="ExternalInput")
    if mask_u8:
        # host-packed, d-major burn-region mask bytes
        msk = nc.dram_tensor(
            "msku8", [NBIG, NPART, IO_G, G, D, b], U8, kind="ExternalInput"
        )
    else:
        msk = nc.dram_tensor("msk", [ROWS_CORE, T, D], F32, kind="ExternalInput")
    out = nc.dram_tensor("out", [ROWS_CORE, T, D], F32, kind="ExternalOutput")
    if post:
        rampneg = nc.dram_tensor("rampneg", [NPART, post], F32, kind="ExternalInput")

    # row = ci*(128*GIO) + p*GIO + a*G + g : each partition holds GIO
    # consecutive rows, so the output DMA sees large contiguous runs.
    srcv = src[:].rearrange("(c p a g) t d -> c p a g (t d)", p=NPART, a=IO_G, g=G)
    outv = out[:].rearrange("(c p a g) t d -> c p a g (t d)", p=NPART, a=IO_G, g=G)
    if mask_u8:
        mskv = msk[:]
    else:
        mskv = msk[:].rearrange(
            "(c p a g) t d -> c p a g (t d)", p=NPART, a=IO_G, g=G
        )

    with TileContext(nc) as tc:
        with (
            tc.tile_pool(name="const", bufs=1) as cpool,
            tc.tile_pool(name="io", bufs=io_bufs) as iop,
            tc.tile_pool(name="wk", bufs=wk_bufs) as wkp,
        ):
            if post:
                ramp_t = cpool.tile([NPART, post], F32, name="ramp_t")
                nc.sync.dma_start(out=ramp_t, in_=rampneg[:])

            s_big = m_big = y_big = None
            for c in range(NCHUNK):
                ci, cs = divmod(c, IO_G)
                if cs == 0:
                    s_big = iop.tile([NPART, IO_G, G, 2 + cb], F32, name="s_ext")
                    if mask_u8:
                        m_big = iop.tile([NPART, IO_G, G, D, b], U8, name="m_t")
                    else:
                        m_big = iop.tile([NPART, IO_G, G, cb], F32, name="m_t")
                    y_big = iop.tile([NPART, IO_G, G, cf], F32, name="y")
                    if mask_u8:
                        # mask first (it heads the compute chain), then
                        # per-sub-chunk src DMAs for finer availability
                        nc.sync.dma_start(out=m_big, in_=mskv[ci])
                        for j in range(IO_G):
                            nc.sync.dma_start(
                                out=s_big[:, j : j + 1, :, 2:],
                                in_=srcv[ci, :, j : j + 1, :, 0:cb],
                            )
                        # no lead-column memset: in the u8 path cols 0:2 are
                        # never read (dx/nbt computed on t>=1; t=0 patched)
                    else:
                        nc.sync.dma_start(
                            out=s_big[:, :, :, 2:], in_=srcv[ci, :, :, :, 0:cb]
                        )
                        nc.sync.dma_start(out=m_big, in_=mskv[ci, :, :, :, 0:cb])
                        nc.vector.memset(s_big[:, :, :, 0:2], 0.0)  # src_{-1}=0
                s_ext = s_big[:, cs]
                m_t = m_big[:, cs]
                y = y_big[:, cs]

                # d-major intermediates: [p, g, d, t]
                app = wkp.tile([NPART, G, D, b], F32, name="app")
                dx = wkp.tile([NPART, G, D, b], F32, name="dx")
                nbt = wkp.tile([NPART, G, D, b], F32, name="nbt")
                nv = wkp.tile([NPART, G, D, b], F32, name="nv")

                s_hi4 = s_ext[:, :, 2:].rearrange("p g (t d) -> p g d t", d=D)
                s_lo4 = s_ext[:, :, 0:cb].rearrange("p g (t d) -> p g d t", d=D)

                if mask_u8:
                    # app_t = m_{t-1} < m_t for t>=1 (binary (1-m_prev)*m_t);
                    # col 0 stays 0 for the self-initializing scan.
                    nc.vector.memset(app[:, :, :, 0:1], 0.0)
                    if b > 1:
                        nc.vector.tensor_tensor(
                            app[:, :, :, 1:],
                            m_t[:, :, :, 0 : b - 1],
                            m_t[:, :, :, 1:],
                            Alu.is_lt,
                        )
                    # dx/nbt only for t>=1 (t=0 handled by the nbt_0 patch;
                    # the unread lead cols of s_ext then need no memset)
                    if b > 1:
                        nc.gpsimd.tensor_tensor(
                            dx[:, :, :, 1:],
                            s_hi4[:, :, :, 1:],
                            s_lo4[:, :, :, 1:],
                            Alu.subtract,
                        )
                        nc.vector.scalar_tensor_tensor(
                            nbt[:, :, :, 1:],
                            app[:, :, :, 1:],
                            1.0,
                            dx[:, :, :, 1:],
                            Alu.subtract,
                            Alu.mult,
                        )
                    # true nbt_0 = (m_0 - 1) * src_0 (app_0 = m_0, v_{-1} = 0)
                    nc.vector.scalar_tensor_tensor(
                        nbt[:, :, :, 0:1],
                        m_t[:, :, :, 0:1],
                        1.0,
                        s_hi4[:, :, :, 0:1],
                        Alu.subtract,
                        Alu.mult,
                    )
                else:
                    omm = wkp.tile([NPART, G, D, 1 + b], F32, name="omm")
                    m4 = m_t.rearrange("p g (t d) -> p g d t", d=D)
                    nc.gpsimd.memset(omm[:, :, :, 0:1], 1.0)
                    nc.gpsimd.tensor_scalar(
                        omm[:, :, :, 1:], m4, -1.0, 1.0, Alu.mult, Alu.add
                    )
                    nc.gpsimd.tensor_tensor(app, omm[:, :, :, 0:b], m4, Alu.mult)
                    # dx = src_t - src_{t-1} (Pool)
                    nc.gpsimd.tensor_tensor(dx, s_hi4, s_lo4, Alu.subtract)
                    # nbt = (app - 1) * dx = -dx*(1-app)
                    nc.vector.scalar_tensor_tensor(
                        nbt, app, 1.0, dx, Alu.subtract, Alu.mult
                    )
                    # zero each sequence's first multiplier after nbt read it
                    nc.vector.memset(app[:, :, :, 0:1], 0.0)

                # single scan across all (g, d) sequences: nv = -v
                nc.vector.tensor_tensor_scan(
                    nv.rearrange("p g d t -> p (g d t)"),
                    app.rearrange("p g d t -> p (g d t)"),
                    nbt.rearrange("p g d t -> p (g d t)"),
                    0.0,
                    Alu.mult,
                    Alu.add,
                )

                # y_burn = src + v = src - nv (3 groups DVE, 1 group Pool).
                # The t=b-1 column goes first as a tiny separate op so the
                # ACT post ops (which only need y_{b-1}) unblock early.
                y4 = y[:, :, 0:cb].rearrange("p g (t d) -> p g t d", d=D)
                sh4 = s_ext[:, :, 2:].rearrange("p g (t d) -> p g t d", d=D)
                nv4 = nv.rearrange("p g d t -> p g t d")
                ky = G - 1
                tlo = b - 1 if post else b
                if post:
                    nc.vector.tensor_tensor(
                        y4[:, :, b - 1 :],
                        sh4[:, :, b - 1 :],
                        nv4[:, :, b - 1 :],
                        Alu.subtract,
                    )
                nc.vector.tensor_tensor(
                    y4[:, 0:ky, 0:tlo], sh4[:, 0:ky, 0:tlo],
                    nv4[:, 0:ky, 0:tlo], Alu.subtract
                )
                nc.gpsimd.tensor_tensor(
                    y4[:, ky:, 0:tlo], sh4[:, ky:, 0:tlo],
                    nv4[:, ky:, 0:tlo], Alu.subtract
                )

                if post:
                    # y_post[k] = y_{b-1} + (k+1)*v_{b-1}
                    #           = rampneg[k]*nv_{b-1} + y_{b-1}   (ACT)
                    for gg in range(G):
                        for d in range(D):
                            nv1 = nv[:, gg, d, b - 1 : b]
                            y1 = y[:, gg, cb - 2 + d : cb - 1 + d]
                            dst = y[:, gg, cb + d : cf : 2]
                            nc.scalar.activation(
                                dst, ramp_t, Act.Identity, bias=y1, scale=nv1
                            )
                    nc.sync.dma_start(out=outv[ci, :, cs, :, 0:cb], in_=y[:, :, 0:cb])
                    nc.sync.dma_start(out=outv[ci, :, cs, :, cb:], in_=y[:, :, cb:])
                else:
                    nc.sync.dma_start(out=outv[ci, :, cs], in_=y)
    nc.compile()
    return nc


_NC_CACHE: dict = {}


def kernel(source, mask, A=None, B=None, C=None, burn_in_steps=64, **_):
    global last_results
    source = np.ascontiguousarray(np.asarray(source, dtype=np.float32))
    mask = np.asarray(mask, dtype=np.float32)
    assert source.shape == (N, T, D), source.shape
    assert mask.shape == (N, T, D), mask.shape

    b = int(burn_in_steps)
    b_eff = T if b <= 0 else min(b, T)
    post = T - b_eff
    NBIG = ROWS_CORE // (NPART * GIO)

    mask_burn = mask[:, :b_eff, :]
    mask_u8 = bool(((mask_burn == 0.0) | (mask_burn == 1.0)).all())

    key = (b_eff, mask_u8)
    if key not in _NC_CACHE:
        _NC_CACHE[key] = _build(b_eff, mask_u8)
    nc = _NC_CACHE[key]

    if mask_u8:
        # pack burn-region mask as d-major uint8 in the kernel's chunk
        # layout: [NCORES*NBIG, NPART, GIO, D, b] contiguous
        mu8 = np.ascontiguousarray(mask_burn.transpose(0, 2, 1)).astype(np.uint8)
        mu8 = mu8.reshape(NCORES, NBIG, NPART, GIO, D, b_eff)
    else:
        mask_f = np.ascontiguousarray(mask)

    if post:
        ramp = -np.broadcast_to(
            np.arange(1, post + 1, dtype=np.float32), (NPART, post)
        ).copy()

    in_maps = []
    for c in range(NCORES):
        m = {"src": source[c * ROWS_CORE : (c + 1) * ROWS_CORE]}
        if mask_u8:
            m["msku8"] = mu8[c]
        else:
            m["msk"] = mask_f[c * ROWS_CORE : (c + 1) * ROWS_CORE]
        if post:
            m["rampneg"] = ramp
        in_maps.append(m)

    res = run_bass_kernel_spmd(nc, in_maps, core_ids=list(range(NCORES)))
    last_results = res
    return np.concatenate([r["out"] for r in res.results], axis=0)


# revision 19
# speedup vs baseline: 1.1113x; 1.0061x over previous
"""Trainium2 Bass kernel for the Inertia model (nn_Net_55224689492388).

Math (exact restructuring of the reference scan; per (row n, channel d)):

  burn-in (t < b):
    app_t = (1 - mask_{t-1}) * mask_t        (mask_{-1} = 0)
    dx_t  = src_t - src_{t-1}                (src_{-1} = 0)
    v_t   = app_t * v_{t-1} + dx_t * (1 - app_t)
    y_t   = src_t + v_t
  post (t >= b): v stays constant (x_t - prev_x collapses to v_{t-1}), so
    y_t   = y_{b-1} + (t - b + 1) * v_{b-1}

Only v is sequential - a first-order linear recurrence computed with the
DVE TensorTensorScan instruction; everything else is bulk elementwise.
The kernel loads only the first b timesteps of src/mask (later steps
cannot affect the output) and writes the full output.

Implementation notes:
- Sign trick: nbt = (app - 1) * dx (one scalar_tensor_tensor op) makes
  the scan compute nv = -v, so y_burn = src - nv and the post phase uses
  a host-provided ramp of -(k+1): y_post = rampneg * nv_b1 + y_b1.
- Scan batching: zeroing the scan multiplier at each sequence's first
  element makes the scan self-initializing (v_0 = 0 * carry + nbt_0), so
  a single scan instruction covers every (group, channel) sequence of a
  chunk; intermediates are channel-deinterleaved (d-major) making it one
  contiguous stride-1 pass. The true app_0 enters through nbt_0, patched
  by a tiny one-column op.
- The binary mask ({0,1} from randint) travels as uint8, host-packed
  d-major so each partition's chunk slice is one contiguous >=512B DMA
  run; app is then a single is_lt compare. Falls back to f32 mask loads
  and the general (1-m_prev)*m_t arithmetic if the mask is non-binary.
- Engines: DVE does app/nbt/scan and most of y_burn; Pool does dx and
  the rest of y_burn; ACT does the post-phase extrapolation; all DMA on
  the SP HWDGE queue, output split burn/post for finer overlap.

Sharding: pure data parallel - 65536 rows split as 8192 rows x 8 cores,
no cross-core communication.
"""

import numpy as np

import concourse.bacc as bacc
import concourse.mybir as mybir
from concourse.bass_utils import run_bass_kernel_spmd
from concourse.tile import TileContext

N, T, D = 65536, 128, 2
NCORES = 8
NPART = 128
ROWS_CORE = N // NCORES  # 8192
G = 4  # row-groups per partition per compute chunk
IO_G = 2  # io tiles/DMAs cover IO_G*G groups
GIO = IO_G * G

F32 = mybir.dt.float32
U8 = mybir.dt.uint8
Alu = mybir.AluOpType
Act = mybir.ActivationFunctionType

# Stash of the most recent BassKernelResults (for test.py profiling).
last_results = None


def _pick_bufs(b, mask_u8):
    """Largest (io_bufs, wk_bufs) <= (9, 6) fitting the SBUF budget."""
    cb = 2 * b
    io_per = GIO * ((2 + cb) * 4 + (cb if mask_u8 else cb * 4) + 2 * T * 4)
    wk_per = G * cb * 4 * (4 if mask_u8 else 5)  # app,dx,nbt,nv (+omm)
    budget = 180 * 1024
    io_bufs, wk_bufs = 9, 7
    while io_bufs > 2 and io_bufs * io_per + wk_bufs * wk_per > budget:
        io_bufs -= 1
        if wk_bufs > 2:
            wk_bufs -= 1
    return io_bufs, wk_bufs


def _build(b, mask_u8=True):
    """Build the per-core Bass module for effective burn-in b (1..T)."""
    NCHUNK = ROWS_CORE // (NPART * G)
    NBIG = ROWS_CORE // (NPART * GIO)
    post = T - b
    cb = 2 * b  # burn-region columns (t-major, d-interleaved)
    cf = 2 * T  # full row columns
    io_bufs, wk_bufs = _pick_bufs(b, mask_u8)

    nc = bacc.Bacc("TRN2", target_bir_lowering=False, debug=False)
    src = nc.dram_tensor("src", [ROWS_CORE, T, D], F32, kind="ExternalInput")
    if mask_u8:
        # host-packed, d-major burn-region mask bytes
        msk = nc.dram_tensor(
            "msku8", [NBIG, NPART, IO_G, G, D, b], U8, kind="ExternalInput"
        )
    else:
        msk = nc.dram_tensor("msk", [ROWS_CORE, T, D], F32, kind="ExternalInput")
    out = nc.dram_tensor("out", [ROWS_CORE, T, D], F32, kind="ExternalOutput")
    if post:
        rampneg = nc.dram_tensor("rampneg", [NPART, post], F32, kind="ExternalInput")

    # row = ci*(128*GIO) + p*GIO + a*G + g : each partition holds GIO
    # consecutive rows, so the output DMA sees large contiguous runs.
    srcv = src[:].rearrange("(c p a g) t d -> c p a g (t d)", p=NPART, a=IO_G, g=G)
    outv = out[:].rearrange("(c p a g) t d -> c p a g (t d)", p=NPART, a=IO_G, g=G)
    if mask_u8:
        mskv = msk[:]
    else:
        mskv = msk[:].rearrange(
            "(c p a g) t d -> c p a g (t d)", p=NPART, a=IO_G, g=G
        )

    with TileContext(nc) as tc:
        with (
            tc.tile_pool(name="const", bufs=1) as cpool,
            tc.tile_pool(name="io", bufs=io_bufs) as iop,
            tc.tile_pool(name="wk", bufs=wk_bufs) as wkp,
        ):
            if post:
                ramp_t = cpool.tile([NPART, post], F32, name="ramp_t")
                nc.sync.dma_start(out=ramp_t, in_=rampneg[:])

            s_big = m_big = y_big = None
            for c in range(NCHUNK):
                ci, cs = divmod(c, IO_G)
                if cs == 0:
                    s_big = iop.tile([NPART, IO_G, G, 2 + cb], F32, name="s_ext")
                    if mask_u8:
                        m_big = iop.tile([NPART, IO_G, G, D, b], U8, name="m_t")
                    else:
                        m_big = iop.tile([NPART, IO_G, G, cb], F32, name="m_t")
                    y_big = iop.tile([NPART, IO_G, G, cf], F32, name="y")
                    if mask_u8:
                        # mask first (it heads the compute chain), then
                        # per-sub-chunk src DMAs for finer availability
                        nc.sync.dma_start(out=m_big, in_=mskv[ci])
                        for j in range(IO_G):
                            nc.sync.dma_start(
                                out=s_big[:, j : j + 1, :, 2:],
                                in_=srcv[ci, :, j : j + 1, :, 0:cb],
                            )
                        # no lead-column memset: in the u8 path cols 0:2 are
                        # never read (dx/nbt computed on t>=1; t=0 patched)
                    else:
                        nc.sync.dma_start(
                            out=s_big[:, :, :, 2:], in_=srcv[ci, :, :, :, 0:cb]
                        )
                        nc.sync.dma_start(out=m_big, in_=mskv[ci, :, :, :, 0:cb])
                        nc.vector.memset(s_big[:, :, :, 0:2], 0.0)  # src_{-1}=0
                s_ext = s_big[:, cs]
                m_t = m_big[:, cs]
                y = y_big[:, cs]

                # d-major intermediates: [p, g, d, t]
                app = wkp.tile([NPART, G, D, b], F32, name="app")
                dx = wkp.tile([NPART, G, D, b], F32, name="dx")
                nbt = wkp.tile([NPART, G, D, b], F32, name="nbt")
                nv = wkp.tile([NPART, G, D, b], F32, name="nv")

                s_hi4 = s_ext[:, :, 2:].rearrange("p g (t d) -> p g d t", d=D)
                s_lo4 = s_ext[:, :, 0:cb].rearrange("p g (t d) -> p g d t", d=D)

                if mask_u8:
                    # app_t = m_{t-1} < m_t for t>=1 (binary (1-m_prev)*m_t);
                    # col 0 stays 0 for the self-initializing scan.
                    nc.vector.memset(app[:, :, :, 0:1], 0.0)
                    if b > 1:
                        nc.vector.tensor_tensor(
                            app[:, :, :, 1:],
                            m_t[:, :, :, 0 : b - 1],
                            m_t[:, :, :, 1:],
                            Alu.is_lt,
                        )
                    # dx/nbt only for t>=1 (t=0 handled by the nbt_0 patch;
                    # the unread lead cols of s_ext then need no memset)
                    if b > 1:
                        nc.gpsimd.tensor_tensor(
                            dx[:, :, :, 1:],
                            s_hi4[:, :, :, 1:],
                            s_lo4[:, :, :, 1:],
                            Alu.subtract,
                        )
                        nc.vector.scalar_tensor_tensor(
                            nbt[:, :, :, 1:],
                            app[:, :, :, 1:],
                            1.0,
                            dx[:, :, :, 1:],
                            Alu.subtract,
                            Alu.mult,
                        )
                    # true nbt_0 = (m_0 - 1) * src_0 (app_0 = m_0, v_{-1} = 0)
                    nc.vector.scalar_tensor_tensor(
                        nbt[:, :, :, 0:1],
                        m_t[:, :, :, 0:1],
                        1.0,
                        s_hi4[:, :, :, 0:1],
                        Alu.subtract,
                        Alu.mult,
                    )
                else:
                    omm = wkp.tile([NPART, G, D, 1 + b], F32, name="omm")
                    m4 = m_t.rearrange("p g (t d) -> p g d t", d=D)
                    nc.gpsimd.memset(omm[:, :, :, 0:1], 1.0)
                    nc.gpsimd.tensor_scalar(
                        omm[:, :, :, 1:], m4, -1.0, 1.0, Alu.mult, Alu.add
                    )
                    nc.gpsimd.tensor_tensor(app, omm[:, :, :, 0:b], m4, Alu.mult)
                    # dx = src_t - src_{t-1} (Pool)
                    nc.gpsimd.tensor_tensor(dx, s_hi4, s_lo4, Alu.subtract)
                    # nbt = (app - 1) * dx = -dx*(1-app)
                    nc.vector.scalar_tensor_tensor(
                        nbt, app, 1.0, dx, Alu.subtract, Alu.mult
                    )
                    # zero each sequence's first multiplier after nbt read it
                    nc.vector.memset(app[:, :, :, 0:1], 0.0)

                # single scan across all (g, d) sequences: nv = -v
                nc.vector.tensor_tensor_scan(
                    nv.rearrange("p g d t -> p (g d t)"),
                    app.rearrange("p g d t -> p (g d t)"),
                    nbt.rearrange("p g d t -> p (g d t)"),
                    0.0,
                    Alu.mult,
                    Alu.add,
                )

                # y_burn = src + v = src - nv (3 groups DVE, 1 group Pool).
                # The t=b-1 column goes first as a tiny separate op so the
                # ACT post ops (which only need y_{b-1}) unblock early.
                y4 = y[:, :, 0:cb].rearrange("p g (t d) -> p g t d", d=D)
                sh4 = s_ext[:, :, 2:].rearrange("p g (t d) -> p g t d", d=D)
                nv4 = nv.rearrange("p g d t -> p g t d")
                ky = G - 1
                tlo = b - 1 if post else b
                if post:
                    nc.vector.tensor_tensor(
                        y4[:, :, b - 1 :],
                        sh4[:, :, b - 1 :],
                        nv4[:, :, b - 1 :],
                        Alu.subtract,
                    )
                nc.vector.tensor_tensor(
                    y4[:, 0:ky, 0:tlo], sh4[:, 0:ky, 0:tlo],
                    nv4[:, 0:ky, 0:tlo], Alu.subtract
                )
                nc.gpsimd.tensor_tensor(
                    y4[:, ky:, 0:tlo], sh4[:, ky:, 0:tlo],
                    nv4[:, ky:, 0:tlo], Alu.subtract
                )

                if post:
                    # y_post[k] = y_{b-1} + (k+1)*v_{b-1}
                    #           = rampneg[k]*nv_{b-1} + y_{b-1}   (ACT)
                    for gg in range(G):
                        for d in range(D):
                            nv1 = nv[:, gg, d, b - 1 : b]
                            y1 = y[:, gg, cb - 2 + d : cb - 1 + d]
                            dst = y[:, gg, cb + d : cf : 2]
                            nc.scalar.activation(
                                dst, ramp_t, Act.Identity, bias=y1, scale=nv1
                            )
                    # burn half issued from ACT's HWDGE queue: splits DMA
                    # issue across two sequencers (SP carries mask/src/post)
                    nc.scalar.dma_start(
                        out=outv[ci, :, cs, :, 0:cb], in_=y[:, :, 0:cb]
                    )
                    nc.sync.dma_start(out=outv[ci, :, cs, :, cb:], in_=y[:, :, cb:])
                else:
                    nc.sync.dma_start(out=outv[ci, :, cs], in_=y)
    nc.compile()
    return nc


_NC_CACHE: dict = {}


def kernel(source, mask, A=None, B=None, C=None, burn_in_steps=64, **_):
    global last_results
    source = np.ascontiguousarray(np.asarray(source, dtype=np.float32))
    mask = np.asarray(mask, dtype=np.float32)
    assert source.shape == (N, T, D), source.shape
    assert mask.shape == (N, T, D), mask.shape

    b = int(burn_in_steps)
    b_eff = T if b <= 0 else min(b, T)
    post = T - b_eff
    NBIG = ROWS_CORE // (NPART * GIO)

    mask_burn = mask[:, :b_eff, :]
    mask_u8 = bool(((mask_burn == 0.0) | (mask_burn == 1.0)).all())

    key = (b_eff, mask_u8)
    if key not in _NC_CACHE:
        _NC_CACHE[key] = _build(b_eff, mask_u8)
    nc = _NC_CACHE[key]

    if mask_u8:
        # pack burn-region mask as d-major uint8 in the kernel's chunk
        # layout: [NCORES*NBIG, NPART, GIO, D, b] contiguous
        mu8 = np.ascontiguousarray(mask_burn.transpose(0, 2, 1)).astype(np.uint8)
        mu8 = mu8.reshape(NCORES, NBIG, NPART, GIO, D, b_eff)
    else:
        mask_f = np.ascontiguousarray(mask)

    if post:
        ramp = -np.broadcast_to(
            np.arange(1, post + 1, dtype=np.float32), (NPART, post)
        ).copy()

    in_maps = []
    for c in range(NCORES):
        m = {"src": source[c * ROWS_CORE : (c + 1) * ROWS_CORE]}
        if mask_u8:
            m["msku8"] = mu8[c]
        else:
            m["msk"] = mask_f[c * ROWS_CORE : (c + 1) * ROWS_CORE]
        if post:
            m["rampneg"] = ramp
        in_maps.append(m)

    res = run_bass_kernel_spmd(nc, in_maps, core_ids=list(range(NCORES)))
    last_results = res
    return np.concatenate([r["out"] for r in res.results], axis=0)


# revision 21
# speedup vs baseline: 1.1175x; 1.0056x over previous
"""Trainium2 Bass kernel for the Inertia model (nn_Net_55224689492388).

Math (exact restructuring of the reference scan; per (row n, channel d)):

  burn-in (t < b):
    app_t = (1 - mask_{t-1}) * mask_t        (mask_{-1} = 0)
    dx_t  = src_t - src_{t-1}                (src_{-1} = 0)
    v_t   = app_t * v_{t-1} + dx_t * (1 - app_t)
    y_t   = src_t + v_t
  post (t >= b): v stays constant (x_t - prev_x collapses to v_{t-1}), so
    y_t   = y_{b-1} + (t - b + 1) * v_{b-1}

Only v is sequential - a first-order linear recurrence computed with the
DVE TensorTensorScan instruction; everything else is bulk elementwise.
The kernel loads only the first b timesteps of src/mask (later steps
cannot affect the output) and writes the full output.

Implementation notes:
- Sign trick: nbt = (app - 1) * dx (one scalar_tensor_tensor op) makes
  the scan compute nv = -v, so y_burn = src - nv and the post phase uses
  a host-provided ramp of -(k+1): y_post = rampneg * nv_b1 + y_b1.
- Scan batching: zeroing the scan multiplier at each sequence's first
  element makes the scan self-initializing (v_0 = 0 * carry + nbt_0), so
  a single scan instruction covers every (group, channel) sequence of a
  chunk; intermediates are channel-deinterleaved (d-major) making it one
  contiguous stride-1 pass. The true app_0 enters through nbt_0, patched
  by a tiny one-column op.
- The binary mask ({0,1} from randint) travels as uint8, host-packed
  d-major so each partition's chunk slice is one contiguous >=512B DMA
  run; app is then a single is_lt compare. Falls back to f32 mask loads
  and the general (1-m_prev)*m_t arithmetic if the mask is non-binary.
- Engines: DVE does app/nbt/scan and most of y_burn; Pool does dx and
  the rest of y_burn; ACT does the post-phase extrapolation. Output is
  split burn/post; the burn half issues from ACT's HWDGE queue so DMA
  descriptor generation is spread across two sequencers (SP carries
  mask/src/post).

Sharding: pure data parallel - 65536 rows split as 8192 rows x 8 cores,
no cross-core communication.
"""

import numpy as np

import concourse.bacc as bacc
import concourse.mybir as mybir
from concourse.bass_utils import run_bass_kernel_spmd
from concourse.tile import TileContext

N, T, D = 65536, 128, 2
NCORES = 8
NPART = 128
ROWS_CORE = N // NCORES  # 8192
G = 4  # row-groups per partition per compute chunk
IO_G = 2  # io tiles/DMAs cover IO_G*G groups
GIO = IO_G * G

F32 = mybir.dt.float32
U8 = mybir.dt.uint8
Alu = mybir.AluOpType
Act = mybir.ActivationFunctionType

# Stash of the most recent BassKernelResults (for test.py profiling).
last_results = None


def _pick_bufs(b, mask_u8):
    """Largest (io_bufs, wk_bufs) <= (9, 6) fitting the SBUF budget."""
    cb = 2 * b
    io_per = GIO * ((2 + cb) * 4 + (cb if mask_u8 else cb * 4) + 2 * T * 4)
    wk_per = G * cb * 4 * (4 if mask_u8 else 5)  # app,dx,nbt,nv (+omm)
    if mask_u8 and b == 64:
        return 9, 11  # verified to fit; deepest wk buffering scores best
    budget = 180 * 1024
    io_bufs, wk_bufs = 9, 7
    while io_bufs > 2 and io_bufs * io_per + wk_bufs * wk_per > budget:
        io_bufs -= 1
        if wk_bufs > 2:
            wk_bufs -= 1
    return io_bufs, wk_bufs


def _build(b, mask_u8=True):
    """Build the per-core Bass module for effective burn-in b (1..T)."""
    NCHUNK = ROWS_CORE // (NPART * G)
    NBIG = ROWS_CORE // (NPART * GIO)
    post = T - b
    cb = 2 * b  # burn-region columns (t-major, d-interleaved)
    cf = 2 * T  # full row columns
    io_bufs, wk_bufs = _pick_bufs(b, mask_u8)

    nc = bacc.Bacc("TRN2", target_bir_lowering=False, debug=False)
    src = nc.dram_tensor("src", [ROWS_CORE, T, D], F32, kind="ExternalInput")
    if mask_u8:
        # host-packed, d-major burn-region mask bytes
        msk = nc.dram_tensor(
            "msku8", [NBIG, NPART, IO_G, G, D, b], U8, kind="ExternalInput"
        )
    else:
        msk = nc.dram_tensor("msk", [ROWS_CORE, T, D], F32, kind="ExternalInput")
    out = nc.dram_tensor("out", [ROWS_CORE, T, D], F32, kind="ExternalOutput")
    if post:
        rampneg = nc.dram_tensor("rampneg", [NPART, post], F32, kind="ExternalInput")

    # row = ci*(128*GIO) + p*GIO + a*G + g : each partition holds GIO
    # consecutive rows, so the output DMA sees large contiguous runs.
    srcv = src[:].rearrange("(c p a g) t d -> c p a g (t d)", p=NPART, a=IO_G, g=G)
    outv = out[:].rearrange("(c p a g) t d -> c p a g (t d)", p=NPART, a=IO_G, g=G)
    if mask_u8:
        mskv = msk[:]
    else:
        mskv = msk[:].rearrange(
            "(c p a g) t d -> c p a g (t d)", p=NPART, a=IO_G, g=G
        )

    with TileContext(nc) as tc:
        with (
            tc.tile_pool(name="const", bufs=1) as cpool,
            tc.tile_pool(name="io", bufs=io_bufs) as iop,
            tc.tile_pool(name="wk", bufs=wk_bufs) as wkp,
        ):
            if post:
                ramp_t = cpool.tile([NPART, post], F32, name="ramp_t")
                nc.sync.dma_start(out=ramp_t, in_=rampneg[:])

            s_big = m_big = y_big = None
            for c in range(NCHUNK):
                ci, cs = divmod(c, IO_G)
                if cs == 0:
                    s_big = iop.tile([NPART, IO_G, G, 2 + cb], F32, name="s_ext")
                    if mask_u8:
                        m_big = iop.tile([NPART, IO_G, G, D, b], U8, name="m_t")
                    else:
                        m_big = iop.tile([NPART, IO_G, G, cb], F32, name="m_t")
                    y_big = iop.tile([NPART, IO_G, G, cf], F32, name="y")
                    if mask_u8:
                        # mask first (it heads the compute chain), then
                        # per-sub-chunk src DMAs for finer availability
                        nc.sync.dma_start(out=m_big, in_=mskv[ci])
                        for j in range(IO_G):
                            nc.sync.dma_start(
                                out=s_big[:, j : j + 1, :, 2:],
                                in_=srcv[ci, :, j : j + 1, :, 0:cb],
                            )
                        # no lead-column memset: in the u8 path cols 0:2 are
                        # never read (dx/nbt computed on t>=1; t=0 patched)
                    else:
                        nc.sync.dma_start(
                            out=s_big[:, :, :, 2:], in_=srcv[ci, :, :, :, 0:cb]
                        )
                        nc.sync.dma_start(out=m_big, in_=mskv[ci, :, :, :, 0:cb])
                        nc.vector.memset(s_big[:, :, :, 0:2], 0.0)  # src_{-1}=0
                s_ext = s_big[:, cs]
                m_t = m_big[:, cs]
                y = y_big[:, cs]

                # d-major intermediates: [p, g, d, t]
                app = wkp.tile([NPART, G, D, b], F32, name="app")
                dx = wkp.tile([NPART, G, D, b], F32, name="dx")
                nbt = wkp.tile([NPART, G, D, b], F32, name="nbt")
                nv = wkp.tile([NPART, G, D, b], F32, name="nv")

                s_hi4 = s_ext[:, :, 2:].rearrange("p g (t d) -> p g d t", d=D)
                s_lo4 = s_ext[:, :, 0:cb].rearrange("p g (t d) -> p g d t", d=D)

                if mask_u8:
                    # app_t = m_{t-1} < m_t for t>=1 (binary (1-m_prev)*m_t);
                    # col 0 stays 0 for the self-initializing scan.
                    nc.vector.memset(app[:, :, :, 0:1], 0.0)
                    if b > 1:
                        nc.vector.tensor_tensor(
                            app[:, :, :, 1:],
                            m_t[:, :, :, 0 : b - 1],
                            m_t[:, :, :, 1:],
                            Alu.is_lt,
                        )
                    # dx/nbt only for t>=1 (t=0 handled by the nbt_0 patch;
                    # the unread lead cols of s_ext then need no memset)
                    if b > 1:
                        nc.gpsimd.tensor_tensor(
                            dx[:, :, :, 1:],
                            s_hi4[:, :, :, 1:],
                            s_lo4[:, :, :, 1:],
                            Alu.subtract,
                        )
                        nc.vector.scalar_tensor_tensor(
                            nbt[:, :, :, 1:],
                            app[:, :, :, 1:],
                            1.0,
                            dx[:, :, :, 1:],
                            Alu.subtract,
                            Alu.mult,
                        )
                    # true nbt_0 = (m_0 - 1) * src_0 (app_0 = m_0, v_{-1} = 0)
                    nc.vector.scalar_tensor_tensor(
                        nbt[:, :, :, 0:1],
                        m_t[:, :, :, 0:1],
                        1.0,
                        s_hi4[:, :, :, 0:1],
                        Alu.subtract,
                        Alu.mult,
                    )
                else:
                    omm = wkp.tile([NPART, G, D, 1 + b], F32, name="omm")
                    m4 = m_t.rearrange("p g (t d) -> p g d t", d=D)
                    nc.gpsimd.memset(omm[:, :, :, 0:1], 1.0)
                    nc.gpsimd.tensor_scalar(
                        omm[:, :, :, 1:], m4, -1.0, 1.0, Alu.mult, Alu.add
                    )
                    nc.gpsimd.tensor_tensor(app, omm[:, :, :, 0:b], m4, Alu.mult)
                    # dx = src_t - src_{t-1} (Pool)
                    nc.gpsimd.tensor_tensor(dx, s_hi4, s_lo4, Alu.subtract)
                    # nbt = (app - 1) * dx = -dx*(1-app)
                    nc.vector.scalar_tensor_tensor(
                        nbt, app, 1.0, dx, Alu.subtract, Alu.mult
                    )
                    # zero each sequence's first multiplier after nbt read it
                    nc.vector.memset(app[:, :, :, 0:1], 0.0)

                # single scan across all (g, d) sequences: nv = -v
                nc.vector.tensor_tensor_scan(
                    nv.rearrange("p g d t -> p (g d t)"),
                    app.rearrange("p g d t -> p (g d t)"),
                    nbt.rearrange("p g d t -> p (g d t)"),
                    0.0,
                    Alu.mult,
                    Alu.add,
                )

                # y_burn = src + v = src - nv (3 groups DVE, 1 group Pool).
                # The t=b-1 column goes first as a tiny separate op so the
                # ACT post ops (which only need y_{b-1}) unblock early.
                y4 = y[:, :, 0:cb].rearrange("p g (t d) -> p g t d", d=D)
                sh4 = s_ext[:, :, 2:].rearrange("p g (t d) -> p g t d", d=D)
                nv4 = nv.rearrange("p g d t -> p g t d")
                ky = G - 1
                tlo = b - 1 if post else b
                if post:
                    nc.vector.tensor_tensor(
                        y4[:, :, b - 1 :],
                        sh4[:, :, b - 1 :],
                        nv4[:, :, b - 1 :],
                        Alu.subtract,
                    )
                nc.vector.tensor_tensor(
                    y4[:, 0:ky, 0:tlo], sh4[:, 0:ky, 0:tlo],
                    nv4[:, 0:ky, 0:tlo], Alu.subtract
                )
                nc.gpsimd.tensor_tensor(
                    y4[:, ky:, 0:tlo], sh4[:, ky:, 0:tlo],
                    nv4[:, ky:, 0:tlo], Alu.subtract
                )

                if post:
                    # y_post[k] = y_{b-1} + (k+1)*v_{b-1}
                    #           = rampneg[k]*nv_{b-1} + y_{b-1}   (ACT)
                    for gg in range(G):
                        for d in range(D):
                            nv1 = nv[:, gg, d, b - 1 : b]
                            y1 = y[:, gg, cb - 2 + d : cb - 1 + d]
                            dst = y[:, gg, cb + d : cf : 2]
                            nc.scalar.activation(
                                dst, ramp_t, Act.Identity, bias=y1, scale=nv1
                            )
                    # burn half issued from ACT's HWDGE queue: splits DMA
                    # issue across two sequencers (SP carries mask/src/post)
                    nc.scalar.dma_start(
                        out=outv[ci, :, cs, :, 0:cb], in_=y[:, :, 0:cb]
                    )
                    nc.sync.dma_start(out=outv[ci, :, cs, :, cb:], in_=y[:, :, cb:])
                else:
                    nc.sync.dma_start(out=outv[ci, :, cs], in_=y)
    nc.compile()
    return nc


_NC_CACHE: dict = {}


def kernel(source, mask, A=None, B=None, C=None, burn_in_steps=64, **_):
    global last_results
    source = np.ascontiguousarray(np.asarray(source, dtype=np.float32))
    mask = np.asarray(mask, dtype=np.float32)
    assert source.shape == (N, T, D), source.shape
    assert mask.shape == (N, T, D), mask.shape

    b = int(burn_in_steps)
    b_eff = T if b <= 0 else min(b, T)
    post = T - b_eff
    NBIG = ROWS_CORE // (NPART * GIO)

    mask_burn = mask[:, :b_eff, :]
    mask_u8 = bool(((mask_burn == 0.0) | (mask_burn == 1.0)).all())

    key = (b_eff, mask_u8)
    if key not in _NC_CACHE:
        _NC_CACHE[key] = _build(b_eff, mask_u8)
    nc = _NC_CACHE[key]

    if mask_u8:
        # pack burn-region mask as d-major uint8 in the kernel's chunk
        # layout: [NCORES*NBIG, NPART, GIO, D, b] contiguous
        mu8 = np.ascontiguousarray(mask_burn.transpose(0, 2, 1)).astype(np.uint8)
        mu8 = mu8.reshape(NCORES, NBIG, NPART, GIO, D, b_eff)
    else:
        mask_f = np.ascontiguousarray(mask)

    if post:
        ramp = -np.broadcast_to(
            np.arange(1, post + 1, dtype=np.float32), (NPART, post)
        ).copy()

    in_maps = []
    for c in range(NCORES):
        m = {"src": source[c * ROWS_CORE : (c + 1) * ROWS_CORE]}
        if mask_u8:
            m["msku8"] = mu8[c]
        else:
            m["msk"] = mask_f[c * ROWS_CORE : (c + 1) * ROWS_CORE]
        if post:
            m["rampneg"] = ramp
        in_maps.append(m)

    res = run_bass_kernel_spmd(nc, in_maps, core_ids=list(range(NCORES)))
    last_results = res
    return np.concatenate([r["out"] for r in res.results], axis=0)


# revision 22
# speedup vs baseline: 1.1237x; 1.0055x over previous
"""Trainium2 Bass kernel for the Inertia model (nn_Net_55224689492388).

Math (exact restructuring of the reference scan; per (row n, channel d)):

  burn-in (t < b):
    app_t = (1 - mask_{t-1}) * mask_t        (mask_{-1} = 0)
    dx_t  = src_t - src_{t-1}                (src_{-1} = 0)
    v_t   = app_t * v_{t-1} + dx_t * (1 - app_t)
    y_t   = src_t + v_t
  post (t >= b): v stays constant (x_t - prev_x collapses to v_{t-1}), so
    y_t   = y_{b-1} + (t - b + 1) * v_{b-1}

Only v is sequential - a first-order linear recurrence computed with the
DVE TensorTensorScan instruction; everything else is bulk elementwise.
The kernel loads only the first b timesteps of src/mask (later steps
cannot affect the output) and writes the full output.

Implementation notes:
- Sign trick: nbt = (app - 1) * dx (one scalar_tensor_tensor op) makes
  the scan compute nv = -v, so y_burn = src - nv and the post phase uses
  a host-provided ramp of -(k+1): y_post = rampneg * nv_b1 + y_b1.
- Scan batching: zeroing the scan multiplier at each sequence's first
  element makes the scan self-initializing (v_0 = 0 * carry + nbt_0), so
  a single scan instruction covers every (group, channel) sequence of a
  chunk; intermediates are channel-deinterleaved (d-major) making it one
  contiguous stride-1 pass. The true app_0 enters through nbt_0, patched
  by a tiny one-column op.
- The binary mask ({0,1} from randint) travels as uint8, host-packed
  d-major so each partition's chunk slice is one contiguous >=512B DMA
  run; app is then a single is_lt compare. Falls back to f32 mask loads
  and the general (1-m_prev)*m_t arithmetic if the mask is non-binary.
- Engines: DVE does app/nbt/scan and most of y_burn; Pool does dx and
  the rest of y_burn; ACT does the post-phase extrapolation. Output is
  split burn/post; the burn half issues from ACT's HWDGE queue so DMA
  descriptor generation is spread across two sequencers (SP carries
  mask/src/post).

Sharding: pure data parallel - 65536 rows split as 8192 rows x 8 cores,
no cross-core communication.
"""

import numpy as np

import concourse.bacc as bacc
import concourse.mybir as mybir
from concourse.bass_utils import run_bass_kernel_spmd
from concourse.tile import TileContext

N, T, D = 65536, 128, 2
NCORES = 8
NPART = 128
ROWS_CORE = N // NCORES  # 8192
G = 4  # row-groups per partition per compute chunk
IO_G = 2  # io tiles/DMAs cover IO_G*G groups
GIO = IO_G * G

F32 = mybir.dt.float32
U8 = mybir.dt.uint8
Alu = mybir.AluOpType
Act = mybir.ActivationFunctionType

# Stash of the most recent BassKernelResults (for test.py profiling).
last_results = None


def _pick_bufs(b, mask_u8):
    """Largest (io_bufs, wk_bufs) <= (9, 6) fitting the SBUF budget."""
    cb = 2 * b
    io_per = GIO * ((2 + cb) * 4 + (cb if mask_u8 else cb * 4) + 2 * T * 4)
    wk_per = G * cb * 4 * (4 if mask_u8 else 5)  # app,dx,nbt,nv (+omm)
    if mask_u8 and b == 64:
        return 5, 16  # verified to fit; deep wk buffering + lean io prefetch
    budget = 180 * 1024
    io_bufs, wk_bufs = 9, 7
    while io_bufs > 2 and io_bufs * io_per + wk_bufs * wk_per > budget:
        io_bufs -= 1
        if wk_bufs > 2:
            wk_bufs -= 1
    return io_bufs, wk_bufs


def _build(b, mask_u8=True):
    """Build the per-core Bass module for effective burn-in b (1..T)."""
    NCHUNK = ROWS_CORE // (NPART * G)
    NBIG = ROWS_CORE // (NPART * GIO)
    post = T - b
    cb = 2 * b  # burn-region columns (t-major, d-interleaved)
    cf = 2 * T  # full row columns
    io_bufs, wk_bufs = _pick_bufs(b, mask_u8)

    nc = bacc.Bacc("TRN2", target_bir_lowering=False, debug=False)
    src = nc.dram_tensor("src", [ROWS_CORE, T, D], F32, kind="ExternalInput")
    if mask_u8:
        # host-packed, d-major burn-region mask bytes
        msk = nc.dram_tensor(
            "msku8", [NBIG, NPART, IO_G, G, D, b], U8, kind="ExternalInput"
        )
    else:
        msk = nc.dram_tensor("msk", [ROWS_CORE, T, D], F32, kind="ExternalInput")
    out = nc.dram_tensor("out", [ROWS_CORE, T, D], F32, kind="ExternalOutput")
    if post:
        rampneg = nc.dram_tensor("rampneg", [NPART, post], F32, kind="ExternalInput")

    # row = ci*(128*GIO) + p*GIO + a*G + g : each partition holds GIO
    # consecutive rows, so the output DMA sees large contiguous runs.
    srcv = src[:].rearrange("(c p a g) t d -> c p a g (t d)", p=NPART, a=IO_G, g=G)
    outv = out[:].rearrange("(c p a g) t d -> c p a g (t d)", p=NPART, a=IO_G, g=G)
    if mask_u8:
        mskv = msk[:]
    else:
        mskv = msk[:].rearrange(
            "(c p a g) t d -> c p a g (t d)", p=NPART, a=IO_G, g=G
        )

    with TileContext(nc) as tc:
        with (
            tc.tile_pool(name="const", bufs=1) as cpool,
            tc.tile_pool(name="io", bufs=io_bufs) as iop,
            tc.tile_pool(name="wk", bufs=wk_bufs) as wkp,
        ):
            if post:
                ramp_t = cpool.tile([NPART, post], F32, name="ramp_t")
                nc.sync.dma_start(out=ramp_t, in_=rampneg[:])

            s_big = m_big = y_big = None
            for c in range(NCHUNK):
                ci, cs = divmod(c, IO_G)
                if cs == 0:
                    s_big = iop.tile([NPART, IO_G, G, 2 + cb], F32, name="s_ext")
                    if mask_u8:
                        m_big = iop.tile([NPART, IO_G, G, D, b], U8, name="m_t")
                    else:
                        m_big = iop.tile([NPART, IO_G, G, cb], F32, name="m_t")
                    y_big = iop.tile([NPART, IO_G, G, cf], F32, name="y")
                    if mask_u8:
                        # mask first (it heads the compute chain), then
                        # per-sub-chunk src DMAs for finer availability
                        nc.sync.dma_start(out=m_big, in_=mskv[ci])
                        for j in range(IO_G):
                            nc.sync.dma_start(
                                out=s_big[:, j : j + 1, :, 2:],
                                in_=srcv[ci, :, j : j + 1, :, 0:cb],
                            )
                        # no lead-column memset: in the u8 path cols 0:2 are
                        # never read (dx/nbt computed on t>=1; t=0 patched)
                    else:
                        nc.sync.dma_start(
                            out=s_big[:, :, :, 2:], in_=srcv[ci, :, :, :, 0:cb]
                        )
                        nc.sync.dma_start(out=m_big, in_=mskv[ci, :, :, :, 0:cb])
                        nc.vector.memset(s_big[:, :, :, 0:2], 0.0)  # src_{-1}=0
                s_ext = s_big[:, cs]
                m_t = m_big[:, cs]
                y = y_big[:, cs]

                # d-major intermediates: [p, g, d, t]
                app = wkp.tile([NPART, G, D, b], F32, name="app")
                dx = wkp.tile([NPART, G, D, b], F32, name="dx")
                nbt = wkp.tile([NPART, G, D, b], F32, name="nbt")
                nv = wkp.tile([NPART, G, D, b], F32, name="nv")

                s_hi4 = s_ext[:, :, 2:].rearrange("p g (t d) -> p g d t", d=D)
                s_lo4 = s_ext[:, :, 0:cb].rearrange("p g (t d) -> p g d t", d=D)

                if mask_u8:
                    # app_t = m_{t-1} < m_t for t>=1 (binary (1-m_prev)*m_t);
                    # col 0 stays 0 for the self-initializing scan.
                    nc.vector.memset(app[:, :, :, 0:1], 0.0)
                    if b > 1:
                        nc.vector.tensor_tensor(
                            app[:, :, :, 1:],
                            m_t[:, :, :, 0 : b - 1],
                            m_t[:, :, :, 1:],
                            Alu.is_lt,
                        )
                    # dx/nbt only for t>=1 (t=0 handled by the nbt_0 patch;
                    # the unread lead cols of s_ext then need no memset)
                    if b > 1:
                        nc.gpsimd.tensor_tensor(
                            dx[:, :, :, 1:],
                            s_hi4[:, :, :, 1:],
                            s_lo4[:, :, :, 1:],
                            Alu.subtract,
                        )
                        nc.vector.scalar_tensor_tensor(
                            nbt[:, :, :, 1:],
                            app[:, :, :, 1:],
                            1.0,
                            dx[:, :, :, 1:],
                            Alu.subtract,
                            Alu.mult,
                        )
                    # true nbt_0 = (m_0 - 1) * src_0 (app_0 = m_0, v_{-1} = 0)
                    nc.vector.scalar_tensor_tensor(
                        nbt[:, :, :, 0:1],
                        m_t[:, :, :, 0:1],
                        1.0,
                        s_hi4[:, :, :, 0:1],
                        Alu.subtract,
                        Alu.mult,
                    )
                else:
                    omm = wkp.tile([NPART, G, D, 1 + b], F32, name="omm")
                    m4 = m_t.rearrange("p g (t d) -> p g d t", d=D)
                    nc.gpsimd.memset(omm[:, :, :, 0:1], 1.0)
                    nc.gpsimd.tensor_scalar(
                        omm[:, :, :, 1:], m4, -1.0, 1.0, Alu.mult, Alu.add
                    )
                    nc.gpsimd.tensor_tensor(app, omm[:, :, :, 0:b], m4, Alu.mult)
                    # dx = src_t - src_{t-1} (Pool)
                    nc.gpsimd.tensor_tensor(dx, s_hi4, s_lo4, Alu.subtract)
                    # nbt = (app - 1) * dx = -dx*(1-app)
                    nc.vector.scalar_tensor_tensor(
                        nbt, app, 1.0, dx, Alu.subtract, Alu.mult
                    )
                    # zero each sequence's first multiplier after nbt read it
                    nc.vector.memset(app[:, :, :, 0:1], 0.0)

                # single scan across all (g, d) sequences: nv = -v
                nc.vector.tensor_tensor_scan(
                    nv.rearrange("p g d t -> p (g d t)"),
                    app.rearrange("p g d t -> p (g d t)"),
                    nbt.rearrange("p g d t -> p (g d t)"),
                    0.0,
                    Alu.mult,
                    Alu.add,
                )

                # y_burn = src + v = src - nv (3 groups DVE, 1 group Pool).
                # The t=b-1 column goes first as a tiny separate op so the
                # ACT post ops (which only need y_{b-1}) unblock early.
                y4 = y[:, :, 0:cb].rearrange("p g (t d) -> p g t d", d=D)
                sh4 = s_ext[:, :, 2:].rearrange("p g (t d) -> p g t d", d=D)
                nv4 = nv.rearrange("p g d t -> p g t d")
                ky = G - 1
                tlo = b - 1 if post else b
                if post:
                    nc.vector.tensor_tensor(
                        y4[:, :, b - 1 :],
                        sh4[:, :, b - 1 :],
                        nv4[:, :, b - 1 :],
                        Alu.subtract,
                    )
                nc.vector.tensor_tensor(
                    y4[:, 0:ky, 0:tlo], sh4[:, 0:ky, 0:tlo],
                    nv4[:, 0:ky, 0:tlo], Alu.subtract
                )
                nc.gpsimd.tensor_tensor(
                    y4[:, ky:, 0:tlo], sh4[:, ky:, 0:tlo],
                    nv4[:, ky:, 0:tlo], Alu.subtract
                )

                if post:
                    # y_post[k] = y_{b-1} + (k+1)*v_{b-1}
                    #           = rampneg[k]*nv_{b-1} + y_{b-1}   (ACT)
                    for gg in range(G):
                        for d in range(D):
                            nv1 = nv[:, gg, d, b - 1 : b]
                            y1 = y[:, gg, cb - 2 + d : cb - 1 + d]
                            dst = y[:, gg, cb + d : cf : 2]
                            nc.scalar.activation(
                                dst, ramp_t, Act.Identity, bias=y1, scale=nv1
                            )
                    # burn half issued from ACT's HWDGE queue: splits DMA
                    # issue across two sequencers (SP carries mask/src/post)
                    nc.scalar.dma_start(
                        out=outv[ci, :, cs, :, 0:cb], in_=y[:, :, 0:cb]
                    )
                    nc.sync.dma_start(out=outv[ci, :, cs, :, cb:], in_=y[:, :, cb:])
                else:
                    nc.sync.dma_start(out=outv[ci, :, cs], in_=y)
    nc.compile()
    return nc


_NC_CACHE: dict = {}


def kernel(source, mask, A=None, B=None, C=None, burn_in_steps=64, **_):
    global last_results
    source = np.ascontiguousarray(np.asarray(source, dtype=np.float32))
    mask = np.asarray(mask, dtype=np.float32)
    assert source.shape == (N, T, D), source.shape
    assert mask.shape == (N, T, D), mask.shape

    b = int(burn_in_steps)
    b_eff = T if b <= 0 else min(b, T)
    post = T - b_eff
    NBIG = ROWS_CORE // (NPART * GIO)

    mask_burn = mask[:, :b_eff, :]
    mask_u8 = bool(((mask_burn == 0.0) | (mask_burn == 1.0)).all())

    key = (b_eff, mask_u8)
    if key not in _NC_CACHE:
        _NC_CACHE[key] = _build(b_eff, mask_u8)
    nc = _NC_CACHE[key]

    if mask_u8:
        # pack burn-region mask as d-major uint8 in the kernel's chunk
        # layout: [NCORES*NBIG, NPART, GIO, D, b] contiguous
        mu8 = np.ascontiguousarray(mask_burn.transpose(0, 2, 1)).astype(np.uint8)
        mu8 = mu8.reshape(NCORES, NBIG, NPART, GIO, D, b_eff)
    else:
        mask_f = np.ascontiguousarray(mask)

    if post:
        ramp = -np.broadcast_to(
            np.arange(1, post + 1, dtype=np.float32), (NPART, post)
        ).copy()

    in_maps = []
    for c in range(NCORES):
        m = {"src": source[c * ROWS_CORE : (c + 1) * ROWS_CORE]}
        if mask_u8:
            m["msku8"] = mu8[c]
        else:
            m["msk"] = mask_f[c * ROWS_CORE : (c + 1) * ROWS_CORE]
        if post:
            m["rampneg"] = ramp
        in_maps.append(m)

    res = run_bass_kernel_spmd(nc, in_maps, core_ids=list(range(NCORES)))
    last_results = res
    return np.concatenate([r["out"] for r in res.results], axis=0)


# revision 23
# speedup vs baseline: 1.1389x; 1.0135x over previous
"""Trainium2 Bass kernel for the Inertia model (nn_Net_55224689492388).

Math (exact restructuring of the reference scan; per (row n, channel d)):

  burn-in (t < b):
    app_t = (1 - mask_{t-1}) * mask_t        (mask_{-1} = 0)
    dx_t  = src_t - src_{t-1}                (src_{-1} = 0)
    v_t   = app_t * v_{t-1} + dx_t * (1 - app_t)
    y_t   = src_t + v_t
  post (t >= b): v stays constant (x_t - prev_x collapses to v_{t-1}), so
    y_t   = y_{b-1} + (t - b + 1) * v_{b-1}

Only v is sequential - a first-order linear recurrence computed with the
DVE TensorTensorScan instruction; everything else is bulk elementwise.
The kernel loads only the first b timesteps of src/mask (later steps
cannot affect the output) and writes the full output.

Implementation notes:
- Sign trick: nbt = (app - 1) * dx (one scalar_tensor_tensor op) makes
  the scan compute nv = -v, so y_burn = src - nv and the post phase uses
  a host-provided ramp of -(k+1): y_post = rampneg * nv_b1 + y_b1.
- Scan batching: zeroing the scan multiplier at each sequence's first
  element makes the scan self-initializing (v_0 = 0 * carry + nbt_0), so
  a single scan instruction covers every (group, channel) sequence of a
  chunk; intermediates are channel-deinterleaved (d-major) making it one
  contiguous stride-1 pass. The true app_0 enters through nbt_0, patched
  by a tiny one-column op.
- The binary mask ({0,1} from randint) travels as uint8, host-packed
  d-major so each partition's chunk slice is one contiguous >=512B DMA
  run; app is then a single is_lt compare. Falls back to f32 mask loads
  and the general (1-m_prev)*m_t arithmetic if the mask is non-binary.
- Engines: DVE does app/nbt/scan and most of y_burn; Pool does dx and
  the rest of y_burn; ACT does the post-phase extrapolation. Output is
  split burn/post; the burn half issues from ACT's HWDGE queue so DMA
  descriptor generation is spread across two sequencers (SP carries
  mask/src/post).

Sharding: pure data parallel - 65536 rows split as 8192 rows x 8 cores,
no cross-core communication.
"""

import numpy as np

import concourse.bacc as bacc
import concourse.mybir as mybir
from concourse.bass_utils import run_bass_kernel_spmd
from concourse.tile import TileContext

N, T, D = 65536, 128, 2
NCORES = 8
NPART = 128
ROWS_CORE = N // NCORES  # 8192
G = 4  # row-groups per partition per compute chunk
IO_G = 2  # io tiles/DMAs cover IO_G*G groups
GIO = IO_G * G

F32 = mybir.dt.float32
U8 = mybir.dt.uint8
Alu = mybir.AluOpType
Act = mybir.ActivationFunctionType

# Stash of the most recent BassKernelResults (for test.py profiling).
last_results = None


def _pick_bufs(b, mask_u8):
    """Largest (io_bufs, wk_bufs) <= (9, 6) fitting the SBUF budget."""
    cb = 2 * b
    io_per = GIO * ((2 + cb) * 4 + (cb if mask_u8 else cb * 4) + 2 * T * 4)
    wk_per = G * cb * 4 * (4 if mask_u8 else 5)  # app,dx,nbt,nv (+omm)
    if mask_u8 and b == 64:
        return 5, 16  # verified to fit; deep wk buffering + lean io prefetch
    budget = 180 * 1024
    io_bufs, wk_bufs = 9, 7
    while io_bufs > 2 and io_bufs * io_per + wk_bufs * wk_per > budget:
        io_bufs -= 1
        if wk_bufs > 2:
            wk_bufs -= 1
    return io_bufs, wk_bufs


def _build(b, mask_u8=True):
    """Build the per-core Bass module for effective burn-in b (1..T)."""
    NCHUNK = ROWS_CORE // (NPART * G)
    NBIG = ROWS_CORE // (NPART * GIO)
    post = T - b
    cb = 2 * b  # burn-region columns (t-major, d-interleaved)
    cf = 2 * T  # full row columns
    io_bufs, wk_bufs = _pick_bufs(b, mask_u8)

    nc = bacc.Bacc("TRN2", target_bir_lowering=False, debug=False)
    src = nc.dram_tensor("src", [ROWS_CORE, T, D], F32, kind="ExternalInput")
    if mask_u8:
        # host-packed, d-major burn-region mask bytes
        msk = nc.dram_tensor(
            "msku8", [NBIG, NPART, IO_G, G, D, b], U8, kind="ExternalInput"
        )
    else:
        msk = nc.dram_tensor("msk", [ROWS_CORE, T, D], F32, kind="ExternalInput")
    out = nc.dram_tensor("out", [ROWS_CORE, T, D], F32, kind="ExternalOutput")
    if post:
        rampneg = nc.dram_tensor("rampneg", [NPART, post], F32, kind="ExternalInput")

    # row = ci*(128*GIO) + p*GIO + a*G + g : each partition holds GIO
    # consecutive rows, so the output DMA sees large contiguous runs.
    srcv = src[:].rearrange("(c p a g) t d -> c p a g (t d)", p=NPART, a=IO_G, g=G)
    outv = out[:].rearrange("(c p a g) t d -> c p a g (t d)", p=NPART, a=IO_G, g=G)
    if mask_u8:
        mskv = msk[:]
    else:
        mskv = msk[:].rearrange(
            "(c p a g) t d -> c p a g (t d)", p=NPART, a=IO_G, g=G
        )

    with TileContext(nc) as tc:
        with (
            tc.tile_pool(name="const", bufs=1) as cpool,
            tc.tile_pool(name="io", bufs=io_bufs) as iop,
            tc.tile_pool(name="wk", bufs=wk_bufs) as wkp,
        ):
            if post:
                # allocated here; its DMA is emitted after the first chunk's
                # input loads so it doesn't outprioritize them on SP
                ramp_t = cpool.tile([NPART, post], F32, name="ramp_t")

            s_big = m_big = y_big = None
            for c in range(NCHUNK):
                ci, cs = divmod(c, IO_G)
                if cs == 0:
                    s_big = iop.tile([NPART, IO_G, G, 2 + cb], F32, name="s_ext")
                    if mask_u8:
                        m_big = iop.tile([NPART, IO_G, G, D, b], U8, name="m_t")
                    else:
                        m_big = iop.tile([NPART, IO_G, G, cb], F32, name="m_t")
                    y_big = iop.tile([NPART, IO_G, G, cf], F32, name="y")
                    if mask_u8:
                        # mask first (it heads the compute chain), then
                        # per-sub-chunk src DMAs for finer availability
                        nc.sync.dma_start(out=m_big, in_=mskv[ci])
                        for j in range(IO_G):
                            nc.sync.dma_start(
                                out=s_big[:, j : j + 1, :, 2:],
                                in_=srcv[ci, :, j : j + 1, :, 0:cb],
                            )
                        # no lead-column memset: in the u8 path cols 0:2 are
                        # never read (dx/nbt computed on t>=1; t=0 patched)
                    else:
                        nc.sync.dma_start(
                            out=s_big[:, :, :, 2:], in_=srcv[ci, :, :, :, 0:cb]
                        )
                        nc.sync.dma_start(out=m_big, in_=mskv[ci, :, :, :, 0:cb])
                        nc.vector.memset(s_big[:, :, :, 0:2], 0.0)  # src_{-1}=0
                if post and c == 0:
                    nc.sync.dma_start(out=ramp_t, in_=rampneg[:])
                s_ext = s_big[:, cs]
                m_t = m_big[:, cs]
                y = y_big[:, cs]

                # d-major intermediates: [p, g, d, t]
                app = wkp.tile([NPART, G, D, b], F32, name="app")
                dx = wkp.tile([NPART, G, D, b], F32, name="dx")
                nbt = wkp.tile([NPART, G, D, b], F32, name="nbt")
                nv = wkp.tile([NPART, G, D, b], F32, name="nv")

                s_hi4 = s_ext[:, :, 2:].rearrange("p g (t d) -> p g d t", d=D)
                s_lo4 = s_ext[:, :, 0:cb].rearrange("p g (t d) -> p g d t", d=D)

                if mask_u8:
                    # app_t = m_{t-1} < m_t for t>=1 (binary (1-m_prev)*m_t);
                    # col 0 stays 0 for the self-initializing scan.
                    nc.vector.memset(app[:, :, :, 0:1], 0.0)
                    if b > 1:
                        nc.vector.tensor_tensor(
                            app[:, :, :, 1:],
                            m_t[:, :, :, 0 : b - 1],
                            m_t[:, :, :, 1:],
                            Alu.is_lt,
                        )
                    # dx/nbt only for t>=1 (t=0 handled by the nbt_0 patch;
                    # the unread lead cols of s_ext then need no memset)
                    if b > 1:
                        nc.gpsimd.tensor_tensor(
                            dx[:, :, :, 1:],
                            s_hi4[:, :, :, 1:],
                            s_lo4[:, :, :, 1:],
                            Alu.subtract,
                        )
                        nc.vector.scalar_tensor_tensor(
                            nbt[:, :, :, 1:],
                            app[:, :, :, 1:],
                            1.0,
                            dx[:, :, :, 1:],
                            Alu.subtract,
                            Alu.mult,
                        )
                    # true nbt_0 = (m_0 - 1) * src_0 (app_0 = m_0, v_{-1} = 0)
                    nc.vector.scalar_tensor_tensor(
                        nbt[:, :, :, 0:1],
                        m_t[:, :, :, 0:1],
                        1.0,
                        s_hi4[:, :, :, 0:1],
                        Alu.subtract,
                        Alu.mult,
                    )
                else:
                    omm = wkp.tile([NPART, G, D, 1 + b], F32, name="omm")
                    m4 = m_t.rearrange("p g (t d) -> p g d t", d=D)
                    nc.gpsimd.memset(omm[:, :, :, 0:1], 1.0)
                    nc.gpsimd.tensor_scalar(
                        omm[:, :, :, 1:], m4, -1.0, 1.0, Alu.mult, Alu.add
                    )
                    nc.gpsimd.tensor_tensor(app, omm[:, :, :, 0:b], m4, Alu.mult)
                    # dx = src_t - src_{t-1} (Pool)
                    nc.gpsimd.tensor_tensor(dx, s_hi4, s_lo4, Alu.subtract)
                    # nbt = (app - 1) * dx = -dx*(1-app)
                    nc.vector.scalar_tensor_tensor(
                        nbt, app, 1.0, dx, Alu.subtract, Alu.mult
                    )
                    # zero each sequence's first multiplier after nbt read it
                    nc.vector.memset(app[:, :, :, 0:1], 0.0)

                # single scan across all (g, d) sequences: nv = -v
                nc.vector.tensor_tensor_scan(
                    nv.rearrange("p g d t -> p (g d t)"),
                    app.rearrange("p g d t -> p (g d t)"),
                    nbt.rearrange("p g d t -> p (g d t)"),
                    0.0,
                    Alu.mult,
                    Alu.add,
                )

                # y_burn = src + v = src - nv (3 groups DVE, 1 group Pool).
                # The t=b-1 column goes first as a tiny separate op so the
                # ACT post ops (which only need y_{b-1}) unblock early.
                y4 = y[:, :, 0:cb].rearrange("p g (t d) -> p g t d", d=D)
                sh4 = s_ext[:, :, 2:].rearrange("p g (t d) -> p g t d", d=D)
                nv4 = nv.rearrange("p g d t -> p g t d")
                ky = G - 1
                tlo = b - 1 if post else b
                if post:
                    nc.vector.tensor_tensor(
                        y4[:, :, b - 1 :],
                        sh4[:, :, b - 1 :],
                        nv4[:, :, b - 1 :],
                        Alu.subtract,
                    )
                nc.vector.tensor_tensor(
                    y4[:, 0:ky, 0:tlo], sh4[:, 0:ky, 0:tlo],
                    nv4[:, 0:ky, 0:tlo], Alu.subtract
                )
                nc.gpsimd.tensor_tensor(
                    y4[:, ky:, 0:tlo], sh4[:, ky:, 0:tlo],
                    nv4[:, ky:, 0:tlo], Alu.subtract
                )

                if post:
                    # y_post[k] = y_{b-1} + (k+1)*v_{b-1}
                    #           = rampneg[k]*nv_{b-1} + y_{b-1}   (ACT)
                    for gg in range(G):
                        for d in range(D):
                            nv1 = nv[:, gg, d, b - 1 : b]
                            y1 = y[:, gg, cb - 2 + d : cb - 1 + d]
                            dst = y[:, gg, cb + d : cf : 2]
                            nc.scalar.activation(
                                dst, ramp_t, Act.Identity, bias=y1, scale=nv1
                            )
                    # burn half issued from ACT's HWDGE queue: splits DMA
                    # issue across two sequencers (SP carries mask/src/post)
                    nc.scalar.dma_start(
                        out=outv[ci, :, cs, :, 0:cb], in_=y[:, :, 0:cb]
                    )
                    nc.sync.dma_start(out=outv[ci, :, cs, :, cb:], in_=y[:, :, cb:])
                else:
                    nc.sync.dma_start(out=outv[ci, :, cs], in_=y)
    nc.compile()
    return nc


_NC_CACHE: dict = {}


def kernel(source, mask, A=None, B=None, C=None, burn_in_steps=64, **_):
    global last_results
    source = np.ascontiguousarray(np.asarray(source, dtype=np.float32))
    mask = np.asarray(mask, dtype=np.float32)
    assert source.shape == (N, T, D), source.shape
    assert mask.shape == (N, T, D), mask.shape

    b = int(burn_in_steps)
    b_eff = T if b <= 0 else min(b, T)
    post = T - b_eff
    NBIG = ROWS_CORE // (NPART * GIO)

    mask_burn = mask[:, :b_eff, :]
    mask_u8 = bool(((mask_burn == 0.0) | (mask_burn == 1.0)).all())

    key = (b_eff, mask_u8)
    if key not in _NC_CACHE:
        _NC_CACHE[key] = _build(b_eff, mask_u8)
    nc = _NC_CACHE[key]

    if mask_u8:
        # pack burn-region mask as d-major uint8 in the kernel's chunk
        # layout: [NCORES*NBIG, NPART, GIO, D, b] contiguous
        mu8 = np.ascontiguousarray(mask_burn.transpose(0, 2, 1)).astype(np.uint8)
        mu8 = mu8.reshape(NCORES, NBIG, NPART, GIO, D, b_eff)
    else:
        mask_f = np.ascontiguousarray(mask)

    if post:
        ramp = -np.broadcast_to(
            np.arange(1, post + 1, dtype=np.float32), (NPART, post)
        ).copy()

    in_maps = []
    for c in range(NCORES):
        m = {"src": source[c * ROWS_CORE : (c + 1) * ROWS_CORE]}
        if mask_u8:
            m["msku8"] = mu8[c]
        else:
            m["msk"] = mask_f[c * ROWS_CORE : (c + 1) * ROWS_CORE]
        if post:
            m["rampneg"] = ramp
        in_maps.append(m)

    res = run_bass_kernel_spmd(nc, in_maps, core_ids=list(range(NCORES)))
    last_results = res
    return np.concatenate([r["out"] for r in res.results], axis=0)
